# revision 14
# baseline (speedup 1.0000x reference)
"""BiLSTM + CRF Viterbi decode on 8 trn2 NeuronCores (Bass/Tile, SPMD).

Strategy:
  - cores 0-3: forward LSTM over sequence quarters; cores 4-7: backward LSTM
    run as a forward LSTM over the host-reversed sequence. One SPMD program.
  - the serial recurrence is broken with chunked restart: each core runs
    B chunks of length L as a batch with a W-step warmup halo (forget-gate
    contraction makes the halo error ~1e-15 at W=64).
  - input projection xp = [X|mask] @ [W_ih.T; b] in fp32r, staged in DRAM;
    recurrence matmuls in fp16 (stationary h^T via DMA-transpose, moving
    W_hh^T, fp32 PSUM); gate math fp32 on ACT/DVE.
  - feats = h @ W_lin.T with fp16 hi/lo weights; partial feats AllGather'd
    and assembled on every core.
  - Viterbi via normalized max-plus segmented scan (prefix + suffix);
    path[t] = argmax(alpha_t + beta_t), replicated on every core.
"""

import numpy as np

CFG = dict(T=4096, F=2048, HH=1024, B=64, L=16, W=64)
TAGS, START, STOP = 10, 8, 9
NCORES = 8
NEG = -1e9


def _derive(cfg):
    d = dict(cfg)
    d["H4"] = 4 * cfg["HH"]
    d["KH"] = cfg["HH"] // 128
    d["NCH"] = d["H4"] // 512
    d["STEPS"] = cfg["W"] + cfg["L"]
    d["TCORE"] = cfg["T"] // 4
    assert cfg["B"] * cfg["L"] == d["TCORE"]
    d["KAUG"] = cfg["F"] + 128
    d["KA"] = d["KAUG"] // 128
    d["RROWS"] = d["TCORE"] + cfg["W"]
    d["RPAD"] = ((d["RROWS"] + 127) // 128) * 128
    d["RB"] = d["RPAD"] // 128
    d["S"] = cfg["T"] // 128
    return d


# ---------------------------------------------------------------------------
# device program
# ---------------------------------------------------------------------------

def build_nc(cfg):
    import concourse.bacc as bacc
    import concourse.mybir as mybir
    import concourse.tile as tile

    d = _derive(cfg)
    T, F, HH, B, L, W = (cfg[k] for k in ("T", "F", "HH", "B", "L", "W"))
    H4, KH, NCH, STEPS = d["H4"], d["KH"], d["NCH"], d["STEPS"]
    KA, RPAD, RB, S, TCORE = d["KA"], d["RPAD"], d["RB"], d["S"], d["TCORE"]
    dt = mybir.dt
    AF = mybir.ActivationFunctionType
    ALU = mybir.AluOpType
    AX = mybir.AxisListType

    nc = bacc.Bacc(None, target_bir_lowering=False, num_devices=NCORES)

    XT = nc.dram_tensor("xT", [d["KAUG"], RPAD], dt.float32, kind="ExternalInput")
    WIH = nc.dram_tensor("Wih", [d["KAUG"], H4], dt.float32, kind="ExternalInput")
    WHH = nc.dram_tensor("Whh", [HH, H4], dt.float16, kind="ExternalInput")
    WLH = nc.dram_tensor("WlinHi", [HH, TAGS], dt.float16, kind="ExternalInput")
    WLL = nc.dram_tensor("WlinLo", [HH, TAGS], dt.float16, kind="ExternalInput")
    BLIN = nc.dram_tensor("blin", [16, 1], dt.float32, kind="ExternalInput")
    TRT = nc.dram_tensor("transTflat", [1, 100], dt.float32, kind="ExternalInput")
    STV = nc.dram_tensor("stopv", [1, TAGS], dt.float32, kind="ExternalInput")
    IDF = nc.dram_tensor("identflat", [1, 100], dt.float32, kind="ExternalInput")
    IDB = nc.dram_tensor("identB", [cfg["B"], cfg["B"]], dt.float16, kind="ExternalInput")
    SCORE = nc.dram_tensor("score", [1, 1], dt.float32, kind="ExternalOutput")
    BPATH = nc.dram_tensor("best_path", [1, T], dt.int32, kind="ExternalOutput")
    DBGF = nc.dram_tensor("dbg_feats", [TAGS, T], dt.float32, kind="ExternalOutput")

    with tile.TileContext(nc) as tc:
        with tc.tile_pool(name="dram", bufs=1, space="DRAM") as dpool:
            xp_d = dpool.tile([L, RPAD // L, H4], dt.float32, tag="xp_d")
            ag_in = dpool.tile([16, TCORE], dt.float32, tag="ag_in")
            ag_out = dpool.tile([16 * NCORES, TCORE], dt.float32,
                                addr_space="Shared", tag="ag_out")
            ftT_d = dpool.tile([TAGS, T], dt.float32, tag="ftT_d")
            sp_d = dpool.tile([128, 100], dt.float32, tag="sp_d")
            ss_d = dpool.tile([128, 1], dt.float32, tag="ss_d")
            g16_d = dpool.tile([16, 100], dt.float32, tag="g16_d")
            gs16_d = dpool.tile([16, 1], dt.float32, tag="gs16_d")
            e16_d = dpool.tile([16, 100], dt.float32, tag="e16_d")
            es16_d = dpool.tile([16, 1], dt.float32, tag="es16_d")
            grid_d = dpool.tile([128, 100], dt.float32, tag="grid_d")
            grs_d = dpool.tile([128, 1], dt.float32, tag="grs_d")

            # ================= Phase A: xp GEMM =================
            with (
                nc.named_scope("phA"),
                tc.tile_pool(name="sbA", bufs=1) as sba,
                tc.tile_pool(name="wpan", bufs=2) as wpan,
                tc.tile_pool(name="psA", bufs=4, space="PSUM") as psa,
                tc.tile_pool(name="outA", bufs=4) as outa,
            ):
                xt = sba.tile([128, KA, RPAD], dt.float32r, tag="xt")
                nc.sync.dma_start(
                    xt[:],
                    XT[:].rearrange("(k p) r -> p k r", p=128).bitcast(dt.float32r))
                for j in range(NCH):
                    wp = wpan.tile([128, KA, 512], dt.float32r, tag="wp")
                    nc.sync.dma_start(
                        wp[:],
                        WIH[:].rearrange("(k p) n -> p k n", p=128)
                        [:, :, 512 * j:512 * (j + 1)].bitcast(dt.float32r))
                    for rb in range(RB):
                        pj = psa.tile([128, 512], dt.float32, tag="pj")
                        for k in range(KA):
                            nc.tensor.matmul(pj[:], xt[:, k, 128 * rb:128 * (rb + 1)],
                                             wp[:, k, :], start=(k == 0),
                                             stop=(k == KA - 1))
                        ot = outa.tile([128, 512], dt.float32, tag="ot")
                        nc.vector.tensor_copy(ot[:], pj[:])
                        PPER = 128 // L
                        nc.sync.dma_start(
                            xp_d[:, PPER * rb:PPER * (rb + 1), 512 * j:512 * (j + 1)]
                            .rearrange("m j c -> j m c"), ot[:])

            # ================= Phase B + C =================
            with nc.named_scope("phB"), tc.tile_pool(name="sbB", bufs=1) as sbb:
                whh = sbb.tile([128, KH, H4], dt.float16, tag="whh")
                nc.sync.dma_start(whh[:], WHH[:].rearrange("(k p) n -> p k n", p=128))
                hist = [sbb.tile([128, TCORE], dt.float16, tag=f"hist{k}", name=f"hist{k}")
                        for k in range(KH)]
                halo = [sbb.tile([128, 2 * B], dt.float16, tag=f"halo{k}", name=f"halo{k}")
                        for k in range(KH)]
                zer = sbb.tile([128, B], dt.float16, tag="zer")
                nc.vector.memset(zer[:], 0.0)
                cst = sbb.tile([B, HH], dt.float32, tag="cst")
                nc.vector.memset(cst[:], 0.0)
                sgi = sbb.tile([B, HH], dt.float32, tag="sgi")
                sgf = sbb.tile([B, HH], dt.float32, tag="sgf")
                sgg = sbb.tile([B, HH], dt.float32, tag="sgg")
                sgo = sbb.tile([B, HH], dt.float32, tag="sgo")
                tnc = sbb.tile([B, HH], dt.float32, tag="tnc")
                tm1 = sbb.tile([B, HH], dt.float32, tag="tm1")
                tm2 = sbb.tile([B, HH], dt.float32, tag="tm2")
                h16 = sbb.tile([B, HH], dt.float16, tag="h16")
                gsb = [sbb.tile([B, 512], dt.float32, tag=f"gsb{n}", name=f"gsb{n}") for n in range(NCH)]
                sgate = {0: sgi, 1: sgf, 2: sgg, 3: sgo}

                def hsrc(t, k):
                    if t < 0:
                        return zer[:]
                    if t < W:
                        return halo[k][:, (t % 2) * B:(t % 2 + 1) * B]
                    return hist[k][:, (t - W) * B:(t - W + 1) * B]

                with (
                    tc.tile_pool(name="xqp", bufs=2) as xqp,
                    tc.tile_pool(name="psB", bufs=1, space="PSUM") as psb,
                ):
                    NPG = min(4, NCH)
                    pg = [psb.tile([B, 512], dt.float32, tag=f"pg{n}", name=f"pg{n}")
                          for n in range(NPG)]
                    ptx = [psb.tile([128, 4 * B], dt.float16, tag=f"ptx{i}", name=f"ptx{i}")
                           for i in range(2)]
                    idb = sbb.tile([B, B], dt.float16, tag="idb")
                    nc.sync.dma_start(idb[:], IDB[:])
                    for t in range(STEPS):
                        xq = xqp.tile([B, H4], dt.float32, tag="xq")
                        nc.sync.dma_start(xq[:], xp_d[t % L, t // L:t // L + B, :])
                        for n in range(NCH):
                            for k in range(KH):
                                nc.tensor.matmul(pg[n % NPG][:], hsrc(t - 1, k),
                                                 whh[:, k, 512 * n:512 * (n + 1)],
                                                 start=(k == 0), stop=(k == KH - 1))
                            nc.vector.tensor_tensor(gsb[n][:], pg[n % NPG][:],
                                                    xq[:, 512 * n:512 * (n + 1)],
                                                    ALU.add)
                            c0, c1 = 512 * n, 512 * (n + 1)
                            for g in range(c0 // HH, (c1 - 1) // HH + 1):
                                lo, hi = max(c0, g * HH), min(c1, (g + 1) * HH)
                                fn = AF.Tanh if g == 2 else AF.Sigmoid
                                nc.scalar.activation(
                                    sgate[g][:, lo - g * HH:hi - g * HH],
                                    gsb[n][:, lo - c0:hi - c0], fn)
                        nc.vector.tensor_tensor(tm1[:], sgf[:], cst[:], ALU.mult)
                        nc.vector.tensor_tensor(tm2[:], sgi[:], sgg[:], ALU.mult)
                        nc.vector.tensor_tensor(cst[:], tm1[:], tm2[:], ALU.add)
                        nc.scalar.activation(tnc[:], cst[:], AF.Tanh)
                        nc.vector.tensor_tensor(h16[:], sgo[:], tnc[:], ALU.mult)
                        import os as _os
                        for k in range(KH):
                            dst = (halo[k][:, (t % 2) * B:(t % 2 + 1) * B] if t < W
                                   else hist[k][:, (t - W) * B:(t - W + 1) * B])
                            if _os.environ.get("NO_PE_T"):
                                nc.sync.dma_start_transpose(
                                    dst, h16[:, 128 * k:128 * (k + 1)])
                            else:
                                pslot = ptx[(k // 4) % 2][:, (k % 4) * B:(k % 4 + 1) * B]
                                nc.tensor.transpose(pslot,
                                                    h16[:, 128 * k:128 * (k + 1)],
                                                    idb[:])
                                nc.vector.tensor_copy(dst, pslot)

                # ---- Phase C: partial feats ----
                with (
                    nc.named_scope("phC"),
                    tc.tile_pool(name="sbC", bufs=1) as sbc,
                    tc.tile_pool(name="psC", bufs=2, space="PSUM") as psc,
                ):
                    wlh = sbc.tile([128, KH, TAGS], dt.float16, tag="wlh")
                    wll = sbc.tile([128, KH, TAGS], dt.float16, tag="wll")
                    nc.sync.dma_start(wlh[:], WLH[:].rearrange("(k p) n -> p k n", p=128))
                    nc.sync.dma_start(wll[:], WLL[:].rearrange("(k p) n -> p k n", p=128))
                    fpart = sbc.tile([16, TCORE], dt.float32, tag="fpart")
                    nc.vector.memset(fpart[:], 0.0)
                    FW = min(512, TCORE)
                    for n2 in range(TCORE // FW):
                        pf = psc.tile([TAGS, FW], dt.float32, tag="pf")
                        first = True
                        for k in range(KH):
                            for wl in (wlh, wll):
                                nc.tensor.matmul(
                                    pf[:], wl[:, k, :],
                                    hist[k][:, FW * n2:FW * (n2 + 1)],
                                    start=first,
                                    stop=(k == KH - 1 and wl is wll))
                                first = False
                        nc.vector.tensor_copy(
                            fpart[0:TAGS, FW * n2:FW * (n2 + 1)], pf[:])
                    nc.sync.dma_start(ag_in[:], fpart[:])

            nc.gpsimd.collective_compute(
                "AllGather", mybir.AluOpType.bypass,
                replica_groups=[list(range(NCORES))],
                ins=[ag_in[:].opt()], outs=[ag_out[:].opt()])

            # ================= Phase D: assemble feats =================
            with nc.named_scope("phD"), tc.tile_pool(name="sbD", bufs=1) as sbd:
                ag_c = [sbd.tile([16, TCORE], dt.float32, tag=f"ag{c}", name=f"ag{c}")
                        for c in range(NCORES)]
                for c in range(NCORES):
                    nc.sync.dma_start(ag_c[c][:], ag_out[16 * c:16 * (c + 1), :])
                blin_sb = sbd.tile([16, 1], dt.float32, tag="blin_sb")
                nc.sync.dma_start(blin_sb[:], BLIN[:])
                ftT = sbd.tile([TAGS, T], dt.float32, tag="ftT")
                for q in range(4):
                    fwd = (ag_c[q][0:TAGS, :]
                           .rearrange("p (t b) -> p b t", b=B))
                    bwd = (ag_c[7 - q][0:TAGS, ::-1]
                           .rearrange("p (t b) -> p b t", b=B))
                    nc.vector.tensor_tensor(
                        ftT[:, TCORE * q:TCORE * (q + 1)]
                        .rearrange("p (b t) -> p b t", b=B),
                        fwd, bwd, ALU.add)
                nc.vector.tensor_scalar_add(ftT[:], ftT[:], blin_sb[0:TAGS, 0:1])
                nc.sync.dma_start(DBGF[:], ftT[:])
                nc.sync.dma_start(ftT_d[:], ftT[:])

            # ================= Phase E: Viterbi =================
            with nc.named_scope("phE"), tc.tile_pool(name="sbE", bufs=1) as sbe:
                trt = sbe.tile([128, 100], dt.float32, tag="trt")
                stv = sbe.tile([128, TAGS], dt.float32, tag="stv")
                idf = sbe.tile([16, 100], dt.float32, tag="idf")
                zro = sbe.tile([16, 1], dt.float32, tag="zro")
                nc.vector.memset(zro[:], 0.0)
                nc.sync.dma_start(trt[0:1, :], TRT[:])
                nc.sync.dma_start(stv[0:1, :], STV[:])
                nc.sync.dma_start(idf[0:1, :], IDF[:])
                p = 1
                while p < 128:
                    q = min(p, 128 - p)
                    nc.sync.dma_start(trt[p:p + q, :], trt[0:q, :])
                    nc.sync.dma_start(stv[p:p + q, :], stv[0:q, :])
                    if p < 16:
                        q2 = min(p, 16 - p)
                        nc.sync.dma_start(idf[p:p + q2, :], idf[0:q2, :])
                    p *= 2

                ftseg = sbe.tile([128, TAGS, S], dt.float32, tag="ftseg")
                nc.sync.dma_start(ftseg[:],
                                  ftT_d[:].rearrange("n (q r) -> q n r", r=S))
                leaf = sbe.tile([128, S * 100], dt.float32, tag="leaf")
                nc.vector.tensor_tensor(
                    leaf[:].rearrange("q (r i n) -> q r i n", i=TAGS, n=TAGS),
                    trt[:].rearrange("q (i n) -> q i n", i=TAGS)
                    .unsqueeze(1).broadcast_to([128, S, TAGS, TAGS]),
                    ftseg[:].rearrange("q n r -> q r n")
                    .unsqueeze(2).broadcast_to([128, S, TAGS, TAGS]),
                    ALU.add)

                csc = sbe.tile([128, 1000], dt.float32, tag="csc")
                mx1 = sbe.tile([128, 1], dt.float32, tag="mx1")

                def compose(av, bv, ov, shin, shout, P=128):
                    nc.vector.tensor_tensor(
                        csc[0:P, :].rearrange("q (i n k) -> q i n k",
                                              i=TAGS, n=TAGS),
                        av.rearrange("q (i k) -> q i k", i=TAGS)
                        .unsqueeze(2).broadcast_to([P, TAGS, TAGS, TAGS]),
                        bv.rearrange("q (k n) -> q n k", k=TAGS)
                        .unsqueeze(1).broadcast_to([P, TAGS, TAGS, TAGS]),
                        ALU.add)
                    nc.vector.tensor_reduce(
                        ov, csc[0:P, :].rearrange("q (in k) -> q in k", k=TAGS),
                        AX.X, ALU.max)
                    nc.vector.tensor_reduce(mx1[0:P, :], ov, AX.X, ALU.max)
                    nc.vector.tensor_scalar_sub(ov, ov, mx1[0:P, 0:1])
                    if len(shin) > 1:
                        nc.vector.tensor_tensor(shout, shin[0], shin[1], ALU.add)
                        nc.vector.tensor_tensor(shout, shout, mx1[0:P, :], ALU.add)
                    else:
                        nc.vector.tensor_tensor(shout, shin[0], mx1[0:P, :], ALU.add)

                def seg_scan(direction, tag):
                    pref = sbe.tile([128, S * 100], dt.float32, tag="pref" + tag)
                    psh = sbe.tile([128, S], dt.float32, tag="psh" + tag)
                    nc.vector.memset(psh[:], 0.0)
                    rng = list(range(S)) if direction > 0 else list(range(S - 1, -1, -1))
                    r0 = rng[0]
                    nc.vector.tensor_copy(pref[:, r0 * 100:(r0 + 1) * 100],
                                          leaf[:, r0 * 100:(r0 + 1) * 100])
                    for r in rng[1:]:
                        prev = r - direction
                        lv = leaf[:, r * 100:(r + 1) * 100]
                        pv = pref[:, prev * 100:(prev + 1) * 100]
                        av, bv = (pv, lv) if direction > 0 else (lv, pv)
                        compose(av, bv, pref[:, r * 100:(r + 1) * 100],
                                [psh[:, prev:prev + 1]], psh[:, r:r + 1])
                    return pref, psh

                def cross_scan(pref, psh, direction, tag):
                    """exclusive scan over the 128 segment products."""
                    last = S - 1 if direction > 0 else 0
                    nc.sync.dma_start(sp_d[:], pref[:, last * 100:(last + 1) * 100])
                    nc.sync.dma_start(ss_d[:], psh[:, last:last + 1])
                    l1 = sbe.tile([16, 800], dt.float32, tag="l1" + tag)
                    l1s = sbe.tile([16, 8], dt.float32, tag="l1s" + tag)
                    nc.sync.dma_start(l1[:], sp_d[:].rearrange("(g j) e -> g (j e)", j=8))
                    nc.sync.dma_start(l1s[:], ss_d[:].rearrange("(g j) e -> g (j e)", j=8))
                    rng = list(range(8)) if direction > 0 else list(range(7, -1, -1))
                    for j in rng[1:]:
                        prev = j - direction
                        sv = l1[:, j * 100:(j + 1) * 100]
                        pv = l1[:, prev * 100:(prev + 1) * 100]
                        av, bv = (pv, sv) if direction > 0 else (sv, pv)
                        compose(av, bv, sv, [l1s[:, prev:prev + 1], l1s[:, j:j + 1]],
                                l1s[:, j:j + 1], P=16)
                    lastj = 7 if direction > 0 else 0
                    nc.sync.dma_start(g16_d[:], l1[:, lastj * 100:(lastj + 1) * 100])
                    nc.sync.dma_start(gs16_d[:], l1s[:, lastj:lastj + 1])
                    l2 = sbe.tile([1, 1600], dt.float32, tag="l2" + tag)
                    l2s = sbe.tile([1, 16], dt.float32, tag="l2s" + tag)
                    nc.sync.dma_start(l2[0:1, :],
                                      g16_d[:].rearrange("g e -> (g e)").unsqueeze(0))
                    nc.sync.dma_start(l2s[0:1, :],
                                      gs16_d[:].rearrange("g e -> (g e)").unsqueeze(0))
                    rng2 = list(range(16)) if direction > 0 else list(range(15, -1, -1))
                    for j in rng2[1:]:
                        prev = j - direction
                        sv = l2[:, j * 100:(j + 1) * 100]
                        pv = l2[:, prev * 100:(prev + 1) * 100]
                        av, bv = (pv, sv) if direction > 0 else (sv, pv)
                        compose(av, bv, sv, [l2s[:, prev:prev + 1], l2s[:, j:j + 1]],
                                l2s[:, j:j + 1], P=1)
                    # exclusive level-2 (group) prefixes -> e16_d (+ shifts)
                    if direction > 0:
                        nc.sync.dma_start(e16_d[1:16, :], l2[0:1, 0:1500])
                        nc.sync.dma_start(es16_d[1:16, :], l2s[0:1, 0:15])
                        nc.sync.dma_start(e16_d[0:1, :], idf[0:1, :])
                        nc.sync.dma_start(es16_d[0:1, :], zro[0:1, :])
                    else:
                        nc.sync.dma_start(e16_d[0:15, :], l2[0:1, 100:1600])
                        nc.sync.dma_start(es16_d[0:15, :], l2s[0:1, 1:16])
                        nc.sync.dma_start(e16_d[15:16, :], idf[0:1, :])
                        nc.sync.dma_start(es16_d[15:16, :], zro[0:1, :])
                    # shifted within-group prefixes -> grid_d (+ shifts)
                    gv = grid_d[:].rearrange("(h s) e -> h s e", s=8)
                    gsv = grs_d[:].rearrange("(h s) e -> h s e", s=8)
                    if direction > 0:
                        nc.sync.dma_start(gv[:, 1:8, :], l1[:, 0:700])
                        nc.sync.dma_start(gsv[:, 1:8, :], l1s[:, 0:7])
                        nc.sync.dma_start(gv[:, 0:1, :], idf[:, :].unsqueeze(1))
                        nc.sync.dma_start(gsv[:, 0:1, :], zro[:, :].unsqueeze(1))
                    else:
                        nc.sync.dma_start(gv[:, 0:7, :], l1[:, 100:800])
                        nc.sync.dma_start(gsv[:, 0:7, :], l1s[:, 1:8])
                        nc.sync.dma_start(gv[:, 7:8, :], idf[:, :].unsqueeze(1))
                        nc.sync.dma_start(gsv[:, 7:8, :], zro[:, :].unsqueeze(1))
                    # materialize exc [128, 100]
                    arow = sbe.tile([128, 100], dt.float32, tag="arow" + tag)
                    ars = sbe.tile([128, 1], dt.float32, tag="ars" + tag)
                    nc.sync.dma_start(
                        arow[:], e16_d[:].unsqueeze(1).broadcast_to([16, 8, 100]))
                    nc.sync.dma_start(
                        ars[:], es16_d[:].unsqueeze(1).broadcast_to([16, 8, 1]))
                    brow = sbe.tile([128, 100], dt.float32, tag="brow" + tag)
                    brs = sbe.tile([128, 1], dt.float32, tag="brs" + tag)
                    nc.sync.dma_start(brow[:], grid_d[:])
                    nc.sync.dma_start(brs[:], grs_d[:])
                    exc = sbe.tile([128, 100], dt.float32, tag="exc" + tag)
                    excs = sbe.tile([128, 1], dt.float32, tag="excs" + tag)
                    av, bv = (arow[:], brow[:]) if direction > 0 else (brow[:], arow[:])
                    compose(av, bv, exc[:], [ars[:], brs[:]], excs[:])
                    return exc, excs

                prefP, pshP = seg_scan(+1, "P")
                excP, excPs = cross_scan(prefP, pshP, +1, "P")
                prefS, pshS = seg_scan(-1, "S")
                excS, excSs = cross_scan(prefS, pshS, -1, "S")

                # ---- alpha: a[t, n] = max_k excP[q][START, k] + prefP[q,r][k, n]
                cbig = sbe.tile([128, S * 100], dt.float32, tag="cbig")
                aseg = sbe.tile([128, S * TAGS], dt.float32, tag="aseg")
                nc.vector.tensor_tensor(
                    cbig[:].rearrange("q (r n k) -> q r n k", n=TAGS, k=TAGS),
                    excP[:, START * TAGS:(START + 1) * TAGS]
                    .unsqueeze(1).unsqueeze(2).broadcast_to([128, S, TAGS, TAGS]),
                    prefP[:].rearrange("q (r k n) -> q r n k", k=TAGS, n=TAGS),
                    ALU.add)
                nc.vector.tensor_reduce(
                    aseg[:], cbig[:].rearrange("q (rn k) -> q rn k", k=TAGS),
                    AX.X, ALU.max)

                # ---- z[q][m] = max_k excS[q][m, k] + stopv[k]
                zq = sbe.tile([128, TAGS], dt.float32, tag="zq")
                nc.vector.tensor_tensor(
                    csc[:, 0:100].rearrange("q (m k) -> q m k", m=TAGS),
                    stv[:].unsqueeze(1).broadcast_to([128, TAGS, TAGS]),
                    excS[:].rearrange("q (m k) -> q m k", m=TAGS),
                    ALU.add)
                nc.vector.tensor_reduce(
                    zq[:], csc[:, 0:100].rearrange("q (m k) -> q m k", m=TAGS),
                    AX.X, ALU.max)

                # ---- beta: b[t, i] = max_m prefS[q, r+1][i, m] + z[q][m]
                bseg = sbe.tile([128, S * TAGS], dt.float32, tag="bseg")
                nc.vector.tensor_tensor(
                    cbig[:, 0:(S - 1) * 100]
                    .rearrange("q (r i m) -> q r i m", i=TAGS, m=TAGS),
                    prefS[:, 100:].rearrange("q (r i m) -> q r i m", i=TAGS, m=TAGS),
                    zq[:].unsqueeze(1).unsqueeze(2)
                    .broadcast_to([128, S - 1, TAGS, TAGS]),
                    ALU.add)
                nc.vector.tensor_reduce(
                    bseg[:, 0:(S - 1) * TAGS],
                    cbig[:, 0:(S - 1) * 100].rearrange("q (ri m) -> q ri m", m=TAGS),
                    AX.X, ALU.max)
                nc.vector.tensor_copy(bseg[:, (S - 1) * TAGS:S * TAGS], zq[:])

                # ---- path = argmax(alpha + beta) ----
                sc = sbe.tile([128, S * TAGS], dt.float32, tag="sc")
                nc.vector.tensor_tensor(sc[:], aseg[:], bseg[:], ALU.add)
                mv8 = sbe.tile([128, 8], dt.float32, tag="mv8")
                mi8 = sbe.tile([128, 8], dt.uint32, tag="mi8")
                path = sbe.tile([128, S], dt.int32, tag="path")
                for r in range(S):
                    nc.vector.max_with_indices(mv8[:], mi8[:],
                                               sc[:, r * TAGS:(r + 1) * TAGS])
                    nc.vector.tensor_copy(path[:, r:r + 1],
                                          mi8[:, 0:1].bitcast(dt.int32))
                nc.sync.dma_start(
                    BPATH[0:1, :].rearrange("p (q r) -> (p q) r", q=128), path[:])

                # ---- score = max(alpha_{T-1} + stopv) + shifts ----
                ts0 = sbe.tile([1, TAGS], dt.float32, tag="ts0")
                tsh = sbe.tile([1, 2], dt.float32, tag="tsh")
                nc.sync.dma_start(ts0[:], aseg[127:128, (S - 1) * TAGS:S * TAGS])
                nc.sync.dma_start(tsh[:, 0:1], excPs[127:128, :])
                nc.sync.dma_start(tsh[:, 1:2], pshP[127:128, S - 1:S])
                ts1 = sbe.tile([1, TAGS], dt.float32, tag="ts1")
                ts2 = sbe.tile([1, 1], dt.float32, tag="ts2")
                nc.vector.tensor_tensor(ts1[:], ts0[:], stv[0:1, :], ALU.add)
                nc.vector.tensor_reduce(ts2[:], ts1[:], AX.X, ALU.max)
                nc.vector.tensor_tensor(ts2[:], ts2[:], tsh[:, 0:1], ALU.add)
                nc.vector.tensor_tensor(ts2[:], ts2[:], tsh[:, 1:2], ALU.add)
                nc.sync.dma_start(SCORE[:], ts2[:])
    nc.compile()
    return nc


# ---------------------------------------------------------------------------
# host side
# ---------------------------------------------------------------------------

def prep_inputs(cfg, sentence, W_ih_f, W_hh_f, b_f, W_ih_b, W_hh_b, b_b,
                W_lin, b_lin, transitions):
    d = _derive(cfg)
    T, F, HH, W = cfg["T"], cfg["F"], cfg["HH"], cfg["W"]
    x = np.ascontiguousarray(sentence[:, 0, :], dtype=np.float32)

    def f16(a):
        return a.astype(np.float16)

    def core_inputs(seq, W_ih, W_hh, b, wl_half):
        outs = []
        waug = np.zeros((d["KAUG"], d["H4"]), np.float32)
        waug[0:F] = W_ih.T.astype(np.float32)
        waug[F] = b.astype(np.float32)
        wlh = f16(wl_half)
        wll = f16(wl_half - wlh.astype(np.float32))
        for q in range(4):
            s0 = q * d["TCORE"] - W
            rows = np.zeros((d["RPAD"], d["KAUG"]), np.float32)
            lo = max(s0, 0)
            rows[lo - s0:d["RROWS"], 0:F] = seq[lo:s0 + d["RROWS"]]
            rows[lo - s0:d["RROWS"], F] = 1.0
            outs.append({
                "xT": np.ascontiguousarray(rows.T),
                "Wih": waug,
                "Whh": np.ascontiguousarray(f16(W_hh.T)),
                "WlinHi": np.ascontiguousarray(wlh),
                "WlinLo": np.ascontiguousarray(wll),
            })
        return outs

    wlf = W_lin[:, 0:HH].T.astype(np.float32)      # [HH, 10]
    wlb = W_lin[:, HH:].T.astype(np.float32)
    cores = (core_inputs(x, W_ih_f, W_hh_f, b_f, wlf)
             + core_inputs(x[::-1].copy(), W_ih_b, W_hh_b, b_b, wlb))

    blin = np.zeros((16, 1), np.float32)
    blin[0:TAGS, 0] = b_lin
    trt = np.ascontiguousarray(
        transitions.T.astype(np.float32).reshape(1, 100))
    stopv = transitions[STOP, :].astype(np.float32).reshape(1, TAGS)
    idf = np.full((TAGS, TAGS), NEG, np.float32)
    np.fill_diagonal(idf, 0.0)
    idf = idf.reshape(1, 100)
    idb = np.eye(cfg["B"], dtype=np.float16)
    for m in cores:
        m["blin"] = blin
        m["transTflat"] = trt
        m["stopv"] = np.ascontiguousarray(stopv)
        m["identflat"] = idf
        m["identB"] = idb
    return cores


def kernel(**inputs):
    from concourse.bass_utils import run_bass_kernel_spmd
    cfg = CFG
    in_maps = prep_inputs(cfg, **{k: np.asarray(v) for k, v in inputs.items()})
    nc = build_nc(cfg)
    res = run_bass_kernel_spmd(nc, in_maps, list(range(NCORES)))
    r0 = res.results[0]
    score = np.float32(r0["score"][0, 0])
    path = r0["best_path"].reshape(-1).astype(np.int32)
    return score, path


# revision 15
# speedup vs baseline: 1.0013x; 1.0013x over previous
"""BiLSTM + CRF Viterbi decode on 8 trn2 NeuronCores (Bass/Tile, SPMD).

Strategy:
  - cores 0-3: forward LSTM over sequence quarters; cores 4-7: backward LSTM
    run as a forward LSTM over the host-reversed sequence. One SPMD program.
  - the serial recurrence is broken with chunked restart: each core runs
    B chunks of length L as a batch with a W-step warmup halo (forget-gate
    contraction makes the halo error ~1e-15 at W=64).
  - input projection xp = [X|mask] @ [W_ih.T; b] in fp32r, staged in DRAM;
    recurrence matmuls in fp16 (stationary h^T via DMA-transpose, moving
    W_hh^T, fp32 PSUM); gate math fp32 on ACT/DVE.
  - feats = h @ W_lin.T with fp16 hi/lo weights; partial feats AllGather'd
    and assembled on every core.
  - Viterbi via normalized max-plus segmented scan (prefix + suffix);
    path[t] = argmax(alpha_t + beta_t), replicated on every core.
"""

import numpy as np

CFG = dict(T=4096, F=2048, HH=1024, B=64, L=16, W=64)
TAGS, START, STOP = 10, 8, 9
NCORES = 8
NEG = -1e9


def _derive(cfg):
    d = dict(cfg)
    d["H4"] = 4 * cfg["HH"]
    d["KH"] = cfg["HH"] // 128
    d["NCH"] = d["H4"] // 512
    d["STEPS"] = cfg["W"] + cfg["L"]
    d["TCORE"] = cfg["T"] // 4
    assert cfg["B"] * cfg["L"] == d["TCORE"]
    d["KAUG"] = cfg["F"] + 128
    d["KA"] = d["KAUG"] // 128
    d["RROWS"] = d["TCORE"] + cfg["W"]
    d["RPAD"] = ((d["RROWS"] + 127) // 128) * 128
    d["RB"] = d["RPAD"] // 128
    d["S"] = cfg["T"] // 128
    return d


# ---------------------------------------------------------------------------
# device program
# ---------------------------------------------------------------------------

def build_nc(cfg):
    import concourse.bacc as bacc
    import concourse.mybir as mybir
    import concourse.tile as tile

    d = _derive(cfg)
    T, F, HH, B, L, W = (cfg[k] for k in ("T", "F", "HH", "B", "L", "W"))
    H4, KH, NCH, STEPS = d["H4"], d["KH"], d["NCH"], d["STEPS"]
    KA, RPAD, RB, S, TCORE = d["KA"], d["RPAD"], d["RB"], d["S"], d["TCORE"]
    dt = mybir.dt
    AF = mybir.ActivationFunctionType
    ALU = mybir.AluOpType
    AX = mybir.AxisListType

    nc = bacc.Bacc(None, target_bir_lowering=False, num_devices=NCORES)

    XT = nc.dram_tensor("xT", [d["KAUG"], RPAD], dt.float32, kind="ExternalInput")
    WIH = nc.dram_tensor("Wih", [d["KAUG"], H4], dt.float32, kind="ExternalInput")
    WHH = nc.dram_tensor("Whh", [HH, H4], dt.float16, kind="ExternalInput")
    WLH = nc.dram_tensor("WlinHi", [HH, TAGS], dt.float16, kind="ExternalInput")
    WLL = nc.dram_tensor("WlinLo", [HH, TAGS], dt.float16, kind="ExternalInput")
    BLIN = nc.dram_tensor("blin", [16, 1], dt.float32, kind="ExternalInput")
    TRT = nc.dram_tensor("transTflat", [1, 100], dt.float32, kind="ExternalInput")
    STV = nc.dram_tensor("stopv", [1, TAGS], dt.float32, kind="ExternalInput")
    IDF = nc.dram_tensor("identflat", [1, 100], dt.float32, kind="ExternalInput")
    IDB = nc.dram_tensor("identB", [cfg["B"], cfg["B"]], dt.float16, kind="ExternalInput")
    SCORE = nc.dram_tensor("score", [1, 1], dt.float32, kind="ExternalOutput")
    BPATH = nc.dram_tensor("best_path", [1, T], dt.int32, kind="ExternalOutput")
    DBGF = nc.dram_tensor("dbg_feats", [TAGS, T], dt.float32, kind="ExternalOutput")

    with tile.TileContext(nc) as tc:
        with tc.tile_pool(name="dram", bufs=1, space="DRAM") as dpool:
            xp_d = dpool.tile([L, RPAD // L, H4], dt.float32, tag="xp_d")
            ag_in = dpool.tile([16, TCORE], dt.float32, tag="ag_in")
            ag_out = dpool.tile([16 * NCORES, TCORE], dt.float32,
                                addr_space="Shared", tag="ag_out")
            ftT_d = dpool.tile([TAGS, T], dt.float32, tag="ftT_d")
            sp_d = dpool.tile([128, 100], dt.float32, tag="sp_d")
            ss_d = dpool.tile([128, 1], dt.float32, tag="ss_d")
            g16_d = dpool.tile([16, 100], dt.float32, tag="g16_d")
            gs16_d = dpool.tile([16, 1], dt.float32, tag="gs16_d")
            e16_d = dpool.tile([16, 100], dt.float32, tag="e16_d")
            es16_d = dpool.tile([16, 1], dt.float32, tag="es16_d")
            grid_d = dpool.tile([128, 100], dt.float32, tag="grid_d")
            grs_d = dpool.tile([128, 1], dt.float32, tag="grs_d")

            # ================= Phase A: xp GEMM =================
            with (
                nc.named_scope("phA"),
                tc.tile_pool(name="sbA", bufs=1) as sba,
                tc.tile_pool(name="wpan", bufs=2) as wpan,
                tc.tile_pool(name="psA", bufs=4, space="PSUM") as psa,
                tc.tile_pool(name="outA", bufs=4) as outa,
            ):
                xt = sba.tile([128, KA, RPAD], dt.float32r, tag="xt")
                nc.sync.dma_start(
                    xt[:],
                    XT[:].rearrange("(k p) r -> p k r", p=128).bitcast(dt.float32r))
                for j in range(NCH):
                    wp = wpan.tile([128, KA, 512], dt.float32r, tag="wp")
                    nc.sync.dma_start(
                        wp[:],
                        WIH[:].rearrange("(k p) n -> p k n", p=128)
                        [:, :, 512 * j:512 * (j + 1)].bitcast(dt.float32r))
                    for rb in range(RB):
                        pj = psa.tile([128, 512], dt.float32, tag="pj")
                        for k in range(KA):
                            nc.tensor.matmul(pj[:], xt[:, k, 128 * rb:128 * (rb + 1)],
                                             wp[:, k, :], start=(k == 0),
                                             stop=(k == KA - 1))
                        ot = outa.tile([128, 512], dt.float32, tag="ot")
                        nc.vector.tensor_copy(ot[:], pj[:])
                        PPER = 128 // L
                        nc.sync.dma_start(
                            xp_d[:, PPER * rb:PPER * (rb + 1), 512 * j:512 * (j + 1)]
                            .rearrange("m j c -> j m c"), ot[:])

            # ================= Phase B + C =================
            with nc.named_scope("phB"), tc.tile_pool(name="sbB", bufs=1) as sbb:
                whh = sbb.tile([128, KH, H4], dt.float16, tag="whh")
                nc.sync.dma_start(whh[:], WHH[:].rearrange("(k p) n -> p k n", p=128))
                hist = [sbb.tile([128, TCORE], dt.float16, tag=f"hist{k}", name=f"hist{k}")
                        for k in range(KH)]
                halo = [sbb.tile([128, 2 * B], dt.float16, tag=f"halo{k}", name=f"halo{k}")
                        for k in range(KH)]
                zer = sbb.tile([128, B], dt.float16, tag="zer")
                nc.vector.memset(zer[:], 0.0)
                cst = sbb.tile([B, HH], dt.float32, tag="cst")
                nc.vector.memset(cst[:], 0.0)
                sgi = sbb.tile([B, HH], dt.float32, tag="sgi")
                sgf = sbb.tile([B, HH], dt.float32, tag="sgf")
                sgg = sbb.tile([B, HH], dt.float32, tag="sgg")
                sgo = sbb.tile([B, HH], dt.float32, tag="sgo")
                tnc = sbb.tile([B, HH], dt.float32, tag="tnc")
                tm1 = sbb.tile([B, HH], dt.float32, tag="tm1")
                tm2 = sbb.tile([B, HH], dt.float32, tag="tm2")
                h16 = sbb.tile([B, HH], dt.float16, tag="h16")
                gsb = [sbb.tile([B, 512], dt.float32, tag=f"gsb{n}", name=f"gsb{n}") for n in range(NCH)]
                sgate = {0: sgi, 1: sgf, 2: sgg, 3: sgo}

                def hsrc(t, k):
                    if t < 0:
                        return zer[:]
                    if t < W:
                        return halo[k][:, (t % 2) * B:(t % 2 + 1) * B]
                    return hist[k][:, (t - W) * B:(t - W + 1) * B]

                with (
                    tc.tile_pool(name="xqp", bufs=2) as xqp,
                    tc.tile_pool(name="psB", bufs=1, space="PSUM") as psb,
                ):
                    import os as _os2
                    NPG = NCH if _os2.environ.get("NPG_FULL") else min(4, NCH)
                    pg = [psb.tile([B, 512], dt.float32, tag=f"pg{n}", name=f"pg{n}")
                          for n in range(NPG)]
                    ptx = ([] if _os2.environ.get("NPG_FULL") else
                           [psb.tile([128, 4 * B], dt.float16, tag=f"ptx{i}", name=f"ptx{i}")
                            for i in range(2)])
                    idb = sbb.tile([B, B], dt.float16, tag="idb")
                    nc.sync.dma_start(idb[:], IDB[:])
                    for t in range(STEPS):
                        xq = xqp.tile([B, H4], dt.float32, tag="xq")
                        nc.sync.dma_start(xq[:], xp_d[t % L, t // L:t // L + B, :])
                        for n in range(NCH):
                            for k in range(KH):
                                nc.tensor.matmul(pg[n % NPG][:], hsrc(t - 1, k),
                                                 whh[:, k, 512 * n:512 * (n + 1)],
                                                 start=(k == 0), stop=(k == KH - 1))
                            nc.vector.tensor_tensor(gsb[n][:], pg[n % NPG][:],
                                                    xq[:, 512 * n:512 * (n + 1)],
                                                    ALU.add)
                            c0, c1 = 512 * n, 512 * (n + 1)
                            for g in range(c0 // HH, (c1 - 1) // HH + 1):
                                lo, hi = max(c0, g * HH), min(c1, (g + 1) * HH)
                                fn = AF.Tanh if g == 2 else AF.Sigmoid
                                nc.scalar.activation(
                                    sgate[g][:, lo - g * HH:hi - g * HH],
                                    gsb[n][:, lo - c0:hi - c0], fn)
                        nc.vector.tensor_tensor(tm1[:], sgf[:], cst[:], ALU.mult)
                        nc.vector.tensor_tensor(tm2[:], sgi[:], sgg[:], ALU.mult)
                        nc.vector.tensor_tensor(cst[:], tm1[:], tm2[:], ALU.add)
                        nc.scalar.activation(tnc[:], cst[:], AF.Tanh)
                        nc.vector.tensor_tensor(h16[:], sgo[:], tnc[:], ALU.mult)
                        import os as _os
                        for k in range(KH):
                            dst = (halo[k][:, (t % 2) * B:(t % 2 + 1) * B] if t < W
                                   else hist[k][:, (t - W) * B:(t - W + 1) * B])
                            if _os.environ.get("NO_PE_T"):
                                nc.sync.dma_start_transpose(
                                    dst, h16[:, 128 * k:128 * (k + 1)])
                            else:
                                pslot = ptx[(k // 4) % 2][:, (k % 4) * B:(k % 4 + 1) * B]
                                nc.tensor.transpose(pslot,
                                                    h16[:, 128 * k:128 * (k + 1)],
                                                    idb[:])
                                nc.vector.tensor_copy(dst, pslot)

                # ---- Phase C: partial feats ----
                with (
                    nc.named_scope("phC"),
                    tc.tile_pool(name="sbC", bufs=1) as sbc,
                    tc.tile_pool(name="psC", bufs=2, space="PSUM") as psc,
                ):
                    wlh = sbc.tile([128, KH, TAGS], dt.float16, tag="wlh")
                    wll = sbc.tile([128, KH, TAGS], dt.float16, tag="wll")
                    nc.sync.dma_start(wlh[:], WLH[:].rearrange("(k p) n -> p k n", p=128))
                    nc.sync.dma_start(wll[:], WLL[:].rearrange("(k p) n -> p k n", p=128))
                    fpart = sbc.tile([16, TCORE], dt.float32, tag="fpart")
                    nc.vector.memset(fpart[:], 0.0)
                    FW = min(512, TCORE)
                    for n2 in range(TCORE // FW):
                        pf = psc.tile([TAGS, FW], dt.float32, tag="pf")
                        first = True
                        for k in range(KH):
                            for wl in (wlh, wll):
                                nc.tensor.matmul(
                                    pf[:], wl[:, k, :],
                                    hist[k][:, FW * n2:FW * (n2 + 1)],
                                    start=first,
                                    stop=(k == KH - 1 and wl is wll))
                                first = False
                        nc.vector.tensor_copy(
                            fpart[0:TAGS, FW * n2:FW * (n2 + 1)], pf[:])
                    nc.sync.dma_start(ag_in[:], fpart[:])

            nc.gpsimd.collective_compute(
                "AllGather", mybir.AluOpType.bypass,
                replica_groups=[list(range(NCORES))],
                ins=[ag_in[:].opt()], outs=[ag_out[:].opt()])

            # ================= Phase D: assemble feats =================
            with nc.named_scope("phD"), tc.tile_pool(name="sbD", bufs=1) as sbd:
                ag_c = [sbd.tile([16, TCORE], dt.float32, tag=f"ag{c}", name=f"ag{c}")
                        for c in range(NCORES)]
                for c in range(NCORES):
                    nc.sync.dma_start(ag_c[c][:], ag_out[16 * c:16 * (c + 1), :])
                blin_sb = sbd.tile([16, 1], dt.float32, tag="blin_sb")
                nc.sync.dma_start(blin_sb[:], BLIN[:])
                ftT = sbd.tile([TAGS, T], dt.float32, tag="ftT")
                for q in range(4):
                    fwd = (ag_c[q][0:TAGS, :]
                           .rearrange("p (t b) -> p b t", b=B))
                    bwd = (ag_c[7 - q][0:TAGS, ::-1]
                           .rearrange("p (t b) -> p b t", b=B))
                    nc.vector.tensor_tensor(
                        ftT[:, TCORE * q:TCORE * (q + 1)]
                        .rearrange("p (b t) -> p b t", b=B),
                        fwd, bwd, ALU.add)
                nc.vector.tensor_scalar_add(ftT[:], ftT[:], blin_sb[0:TAGS, 0:1])
                nc.sync.dma_start(DBGF[:], ftT[:])
                nc.sync.dma_start(ftT_d[:], ftT[:])

            # ================= Phase E: Viterbi =================
            with nc.named_scope("phE"), tc.tile_pool(name="sbE", bufs=1) as sbe:
                trt = sbe.tile([128, 100], dt.float32, tag="trt")
                stv = sbe.tile([128, TAGS], dt.float32, tag="stv")
                idf = sbe.tile([16, 100], dt.float32, tag="idf")
                zro = sbe.tile([16, 1], dt.float32, tag="zro")
                nc.vector.memset(zro[:], 0.0)
                nc.sync.dma_start(trt[0:1, :], TRT[:])
                nc.sync.dma_start(stv[0:1, :], STV[:])
                nc.sync.dma_start(idf[0:1, :], IDF[:])
                p = 1
                while p < 128:
                    q = min(p, 128 - p)
                    nc.sync.dma_start(trt[p:p + q, :], trt[0:q, :])
                    nc.sync.dma_start(stv[p:p + q, :], stv[0:q, :])
                    if p < 16:
                        q2 = min(p, 16 - p)
                        nc.sync.dma_start(idf[p:p + q2, :], idf[0:q2, :])
                    p *= 2

                ftseg = sbe.tile([128, TAGS, S], dt.float32, tag="ftseg")
                nc.sync.dma_start(ftseg[:],
                                  ftT_d[:].rearrange("n (q r) -> q n r", r=S))
                leaf = sbe.tile([128, S * 100], dt.float32, tag="leaf")
                nc.vector.tensor_tensor(
                    leaf[:].rearrange("q (r i n) -> q r i n", i=TAGS, n=TAGS),
                    trt[:].rearrange("q (i n) -> q i n", i=TAGS)
                    .unsqueeze(1).broadcast_to([128, S, TAGS, TAGS]),
                    ftseg[:].rearrange("q n r -> q r n")
                    .unsqueeze(2).broadcast_to([128, S, TAGS, TAGS]),
                    ALU.add)

                csc = sbe.tile([128, 1000], dt.float32, tag="csc")
                mx1 = sbe.tile([128, 1], dt.float32, tag="mx1")

                def compose(av, bv, ov, shin, shout, P=128):
                    nc.vector.tensor_tensor(
                        csc[0:P, :].rearrange("q (i n k) -> q i n k",
                                              i=TAGS, n=TAGS),
                        av.rearrange("q (i k) -> q i k", i=TAGS)
                        .unsqueeze(2).broadcast_to([P, TAGS, TAGS, TAGS]),
                        bv.rearrange("q (k n) -> q n k", k=TAGS)
                        .unsqueeze(1).broadcast_to([P, TAGS, TAGS, TAGS]),
                        ALU.add)
                    nc.vector.tensor_reduce(
                        ov, csc[0:P, :].rearrange("q (in k) -> q in k", k=TAGS),
                        AX.X, ALU.max)
                    nc.vector.tensor_reduce(mx1[0:P, :], ov, AX.X, ALU.max)
                    nc.vector.tensor_scalar_sub(ov, ov, mx1[0:P, 0:1])
                    if len(shin) > 1:
                        nc.vector.tensor_tensor(shout, shin[0], shin[1], ALU.add)
                        nc.vector.tensor_tensor(shout, shout, mx1[0:P, :], ALU.add)
                    else:
                        nc.vector.tensor_tensor(shout, shin[0], mx1[0:P, :], ALU.add)

                def seg_scan(direction, tag):
                    pref = sbe.tile([128, S * 100], dt.float32, tag="pref" + tag)
                    psh = sbe.tile([128, S], dt.float32, tag="psh" + tag)
                    nc.vector.memset(psh[:], 0.0)
                    rng = list(range(S)) if direction > 0 else list(range(S - 1, -1, -1))
                    r0 = rng[0]
                    nc.vector.tensor_copy(pref[:, r0 * 100:(r0 + 1) * 100],
                                          leaf[:, r0 * 100:(r0 + 1) * 100])
                    for r in rng[1:]:
                        prev = r - direction
                        lv = leaf[:, r * 100:(r + 1) * 100]
                        pv = pref[:, prev * 100:(prev + 1) * 100]
                        av, bv = (pv, lv) if direction > 0 else (lv, pv)
                        compose(av, bv, pref[:, r * 100:(r + 1) * 100],
                                [psh[:, prev:prev + 1]], psh[:, r:r + 1])
                    return pref, psh

                def cross_scan(pref, psh, direction, tag):
                    """exclusive scan over the 128 segment products."""
                    last = S - 1 if direction > 0 else 0
                    nc.sync.dma_start(sp_d[:], pref[:, last * 100:(last + 1) * 100])
                    nc.sync.dma_start(ss_d[:], psh[:, last:last + 1])
                    l1 = sbe.tile([16, 800], dt.float32, tag="l1" + tag)
                    l1s = sbe.tile([16, 8], dt.float32, tag="l1s" + tag)
                    nc.sync.dma_start(l1[:], sp_d[:].rearrange("(g j) e -> g (j e)", j=8))
                    nc.sync.dma_start(l1s[:], ss_d[:].rearrange("(g j) e -> g (j e)", j=8))
                    rng = list(range(8)) if direction > 0 else list(range(7, -1, -1))
                    for j in rng[1:]:
                        prev = j - direction
                        sv = l1[:, j * 100:(j + 1) * 100]
                        pv = l1[:, prev * 100:(prev + 1) * 100]
                        av, bv = (pv, sv) if direction > 0 else (sv, pv)
                        compose(av, bv, sv, [l1s[:, prev:prev + 1], l1s[:, j:j + 1]],
                                l1s[:, j:j + 1], P=16)
                    lastj = 7 if direction > 0 else 0
                    nc.sync.dma_start(g16_d[:], l1[:, lastj * 100:(lastj + 1) * 100])
                    nc.sync.dma_start(gs16_d[:], l1s[:, lastj:lastj + 1])
                    l2 = sbe.tile([1, 1600], dt.float32, tag="l2" + tag)
                    l2s = sbe.tile([1, 16], dt.float32, tag="l2s" + tag)
                    nc.sync.dma_start(l2[0:1, :],
                                      g16_d[:].rearrange("g e -> (g e)").unsqueeze(0))
                    nc.sync.dma_start(l2s[0:1, :],
                                      gs16_d[:].rearrange("g e -> (g e)").unsqueeze(0))
                    rng2 = list(range(16)) if direction > 0 else list(range(15, -1, -1))
                    for j in rng2[1:]:
                        prev = j - direction
                        sv = l2[:, j * 100:(j + 1) * 100]
                        pv = l2[:, prev * 100:(prev + 1) * 100]
                        av, bv = (pv, sv) if direction > 0 else (sv, pv)
                        compose(av, bv, sv, [l2s[:, prev:prev + 1], l2s[:, j:j + 1]],
                                l2s[:, j:j + 1], P=1)
                    # exclusive level-2 (group) prefixes -> e16_d (+ shifts)
                    if direction > 0:
                        nc.sync.dma_start(e16_d[1:16, :], l2[0:1, 0:1500])
                        nc.sync.dma_start(es16_d[1:16, :], l2s[0:1, 0:15])
                        nc.sync.dma_start(e16_d[0:1, :], idf[0:1, :])
                        nc.sync.dma_start(es16_d[0:1, :], zro[0:1, :])
                    else:
                        nc.sync.dma_start(e16_d[0:15, :], l2[0:1, 100:1600])
                        nc.sync.dma_start(es16_d[0:15, :], l2s[0:1, 1:16])
                        nc.sync.dma_start(e16_d[15:16, :], idf[0:1, :])
                        nc.sync.dma_start(es16_d[15:16, :], zro[0:1, :])
                    # shifted within-group prefixes -> grid_d (+ shifts)
                    gv = grid_d[:].rearrange("(h s) e -> h s e", s=8)
                    gsv = grs_d[:].rearrange("(h s) e -> h s e", s=8)
                    if direction > 0:
                        nc.sync.dma_start(gv[:, 1:8, :], l1[:, 0:700])
                        nc.sync.dma_start(gsv[:, 1:8, :], l1s[:, 0:7])
                        nc.sync.dma_start(gv[:, 0:1, :], idf[:, :].unsqueeze(1))
                        nc.sync.dma_start(gsv[:, 0:1, :], zro[:, :].unsqueeze(1))
                    else:
                        nc.sync.dma_start(gv[:, 0:7, :], l1[:, 100:800])
                        nc.sync.dma_start(gsv[:, 0:7, :], l1s[:, 1:8])
                        nc.sync.dma_start(gv[:, 7:8, :], idf[:, :].unsqueeze(1))
                        nc.sync.dma_start(gsv[:, 7:8, :], zro[:, :].unsqueeze(1))
                    # materialize exc [128, 100]
                    arow = sbe.tile([128, 100], dt.float32, tag="arow" + tag)
                    ars = sbe.tile([128, 1], dt.float32, tag="ars" + tag)
                    nc.sync.dma_start(
                        arow[:], e16_d[:].unsqueeze(1).broadcast_to([16, 8, 100]))
                    nc.sync.dma_start(
                        ars[:], es16_d[:].unsqueeze(1).broadcast_to([16, 8, 1]))
                    brow = sbe.tile([128, 100], dt.float32, tag="brow" + tag)
                    brs = sbe.tile([128, 1], dt.float32, tag="brs" + tag)
                    nc.sync.dma_start(brow[:], grid_d[:])
                    nc.sync.dma_start(brs[:], grs_d[:])
                    exc = sbe.tile([128, 100], dt.float32, tag="exc" + tag)
                    excs = sbe.tile([128, 1], dt.float32, tag="excs" + tag)
                    av, bv = (arow[:], brow[:]) if direction > 0 else (brow[:], arow[:])
                    compose(av, bv, exc[:], [ars[:], brs[:]], excs[:])
                    return exc, excs

                prefP, pshP = seg_scan(+1, "P")
                excP, excPs = cross_scan(prefP, pshP, +1, "P")
                prefS, pshS = seg_scan(-1, "S")
                excS, excSs = cross_scan(prefS, pshS, -1, "S")

                # ---- alpha: a[t, n] = max_k excP[q][START, k] + prefP[q,r][k, n]
                cbig = sbe.tile([128, S * 100], dt.float32, tag="cbig")
                aseg = sbe.tile([128, S * TAGS], dt.float32, tag="aseg")
                nc.vector.tensor_tensor(
                    cbig[:].rearrange("q (r n k) -> q r n k", n=TAGS, k=TAGS),
                    excP[:, START * TAGS:(START + 1) * TAGS]
                    .unsqueeze(1).unsqueeze(2).broadcast_to([128, S, TAGS, TAGS]),
                    prefP[:].rearrange("q (r k n) -> q r n k", k=TAGS, n=TAGS),
                    ALU.add)
                nc.vector.tensor_reduce(
                    aseg[:], cbig[:].rearrange("q (rn k) -> q rn k", k=TAGS),
                    AX.X, ALU.max)

                # ---- z[q][m] = max_k excS[q][m, k] + stopv[k]
                zq = sbe.tile([128, TAGS], dt.float32, tag="zq")
                nc.vector.tensor_tensor(
                    csc[:, 0:100].rearrange("q (m k) -> q m k", m=TAGS),
                    stv[:].unsqueeze(1).broadcast_to([128, TAGS, TAGS]),
                    excS[:].rearrange("q (m k) -> q m k", m=TAGS),
                    ALU.add)
                nc.vector.tensor_reduce(
                    zq[:], csc[:, 0:100].rearrange("q (m k) -> q m k", m=TAGS),
                    AX.X, ALU.max)

                # ---- beta: b[t, i] = max_m prefS[q, r+1][i, m] + z[q][m]
                bseg = sbe.tile([128, S * TAGS], dt.float32, tag="bseg")
                nc.vector.tensor_tensor(
                    cbig[:, 0:(S - 1) * 100]
                    .rearrange("q (r i m) -> q r i m", i=TAGS, m=TAGS),
                    prefS[:, 100:].rearrange("q (r i m) -> q r i m", i=TAGS, m=TAGS),
                    zq[:].unsqueeze(1).unsqueeze(2)
                    .broadcast_to([128, S - 1, TAGS, TAGS]),
                    ALU.add)
                nc.vector.tensor_reduce(
                    bseg[:, 0:(S - 1) * TAGS],
                    cbig[:, 0:(S - 1) * 100].rearrange("q (ri m) -> q ri m", m=TAGS),
                    AX.X, ALU.max)
                nc.vector.tensor_copy(bseg[:, (S - 1) * TAGS:S * TAGS], zq[:])

                # ---- path = argmax(alpha + beta) ----
                sc = sbe.tile([128, S * TAGS], dt.float32, tag="sc")
                nc.vector.tensor_tensor(sc[:], aseg[:], bseg[:], ALU.add)
                mv8 = sbe.tile([128, 8], dt.float32, tag="mv8")
                mi8 = sbe.tile([128, 8], dt.uint32, tag="mi8")
                path = sbe.tile([128, S], dt.int32, tag="path")
                for r in range(S):
                    nc.vector.max_with_indices(mv8[:], mi8[:],
                                               sc[:, r * TAGS:(r + 1) * TAGS])
                    nc.vector.tensor_copy(path[:, r:r + 1],
                                          mi8[:, 0:1].bitcast(dt.int32))
                nc.sync.dma_start(
                    BPATH[0:1, :].rearrange("p (q r) -> (p q) r", q=128), path[:])

                # ---- score = max(alpha_{T-1} + stopv) + shifts ----
                ts0 = sbe.tile([1, TAGS], dt.float32, tag="ts0")
                tsh = sbe.tile([1, 2], dt.float32, tag="tsh")
                nc.sync.dma_start(ts0[:], aseg[127:128, (S - 1) * TAGS:S * TAGS])
                nc.sync.dma_start(tsh[:, 0:1], excPs[127:128, :])
                nc.sync.dma_start(tsh[:, 1:2], pshP[127:128, S - 1:S])
                ts1 = sbe.tile([1, TAGS], dt.float32, tag="ts1")
                ts2 = sbe.tile([1, 1], dt.float32, tag="ts2")
                nc.vector.tensor_tensor(ts1[:], ts0[:], stv[0:1, :], ALU.add)
                nc.vector.tensor_reduce(ts2[:], ts1[:], AX.X, ALU.max)
                nc.vector.tensor_tensor(ts2[:], ts2[:], tsh[:, 0:1], ALU.add)
                nc.vector.tensor_tensor(ts2[:], ts2[:], tsh[:, 1:2], ALU.add)
                nc.sync.dma_start(SCORE[:], ts2[:])
    nc.compile()
    return nc


# ---------------------------------------------------------------------------
# host side
# ---------------------------------------------------------------------------

def prep_inputs(cfg, sentence, W_ih_f, W_hh_f, b_f, W_ih_b, W_hh_b, b_b,
                W_lin, b_lin, transitions):
    d = _derive(cfg)
    T, F, HH, W = cfg["T"], cfg["F"], cfg["HH"], cfg["W"]
    x = np.ascontiguousarray(sentence[:, 0, :], dtype=np.float32)

    def f16(a):
        return a.astype(np.float16)

    def core_inputs(seq, W_ih, W_hh, b, wl_half):
        outs = []
        waug = np.zeros((d["KAUG"], d["H4"]), np.float32)
        waug[0:F] = W_ih.T.astype(np.float32)
        waug[F] = b.astype(np.float32)
        wlh = f16(wl_half)
        wll = f16(wl_half - wlh.astype(np.float32))
        for q in range(4):
            s0 = q * d["TCORE"] - W
            rows = np.zeros((d["RPAD"], d["KAUG"]), np.float32)
            lo = max(s0, 0)
            rows[lo - s0:d["RROWS"], 0:F] = seq[lo:s0 + d["RROWS"]]
            rows[lo - s0:d["RROWS"], F] = 1.0
            outs.append({
                "xT": np.ascontiguousarray(rows.T),
                "Wih": waug,
                "Whh": np.ascontiguousarray(f16(W_hh.T)),
                "WlinHi": np.ascontiguousarray(wlh),
                "WlinLo": np.ascontiguousarray(wll),
            })
        return outs

    wlf = W_lin[:, 0:HH].T.astype(np.float32)      # [HH, 10]
    wlb = W_lin[:, HH:].T.astype(np.float32)
    cores = (core_inputs(x, W_ih_f, W_hh_f, b_f, wlf)
             + core_inputs(x[::-1].copy(), W_ih_b, W_hh_b, b_b, wlb))

    blin = np.zeros((16, 1), np.float32)
    blin[0:TAGS, 0] = b_lin
    trt = np.ascontiguousarray(
        transitions.T.astype(np.float32).reshape(1, 100))
    stopv = transitions[STOP, :].astype(np.float32).reshape(1, TAGS)
    idf = np.full((TAGS, TAGS), NEG, np.float32)
    np.fill_diagonal(idf, 0.0)
    idf = idf.reshape(1, 100)
    idb = np.eye(cfg["B"], dtype=np.float16)
    for m in cores:
        m["blin"] = blin
        m["transTflat"] = trt
        m["stopv"] = np.ascontiguousarray(stopv)
        m["identflat"] = idf
        m["identB"] = idb
    return cores


def kernel(**inputs):
    from concourse.bass_utils import run_bass_kernel_spmd
    cfg = CFG
    in_maps = prep_inputs(cfg, **{k: np.asarray(v) for k, v in inputs.items()})
    nc = build_nc(cfg)
    res = run_bass_kernel_spmd(nc, in_maps, list(range(NCORES)))
    r0 = res.results[0]
    score = np.float32(r0["score"][0, 0])
    path = r0["best_path"].reshape(-1).astype(np.int32)
    return score, path


# revision 18
# speedup vs baseline: 1.0017x; 1.0004x over previous
"""BiLSTM + CRF Viterbi decode on 8 trn2 NeuronCores (Bass/Tile, SPMD).

Strategy:
  - cores 0-3: forward LSTM over sequence quarters; cores 4-7: backward LSTM
    run as a forward LSTM over the host-reversed sequence. One SPMD program.
  - the serial recurrence is broken with chunked restart: each core runs
    B chunks of length L as a batch with a W-step warmup halo (forget-gate
    contraction makes the halo error ~1e-15 at W=64).
  - input projection xp = [X|mask] @ [W_ih.T; b] in fp32r, staged in DRAM;
    recurrence matmuls in fp16 (stationary h^T via DMA-transpose, moving
    W_hh^T, fp32 PSUM); gate math fp32 on ACT/DVE.
  - feats = h @ W_lin.T with fp16 hi/lo weights; partial feats AllGather'd
    and assembled on every core.
  - Viterbi via normalized max-plus segmented scan (prefix + suffix);
    path[t] = argmax(alpha_t + beta_t), replicated on every core.
"""

import numpy as np

CFG = dict(T=4096, F=2048, HH=1024, B=64, L=16, W=64)
TAGS, START, STOP = 10, 8, 9
NCORES = 8
NEG = -1e9


def _derive(cfg):
    d = dict(cfg)
    d["H4"] = 4 * cfg["HH"]
    d["KH"] = cfg["HH"] // 128
    d["NCH"] = d["H4"] // 512
    d["STEPS"] = cfg["W"] + cfg["L"]
    d["TCORE"] = cfg["T"] // 4
    assert cfg["B"] * cfg["L"] == d["TCORE"]
    d["KAUG"] = cfg["F"] + 128
    d["KA"] = d["KAUG"] // 128
    d["RROWS"] = d["TCORE"] + cfg["W"]
    d["RPAD"] = ((d["RROWS"] + 127) // 128) * 128
    d["RB"] = d["RPAD"] // 128
    d["S"] = cfg["T"] // 128
    return d


# ---------------------------------------------------------------------------
# device program
# ---------------------------------------------------------------------------

def build_nc(cfg):
    import concourse.bacc as bacc
    import concourse.mybir as mybir
    import concourse.tile as tile

    d = _derive(cfg)
    T, F, HH, B, L, W = (cfg[k] for k in ("T", "F", "HH", "B", "L", "W"))
    H4, KH, NCH, STEPS = d["H4"], d["KH"], d["NCH"], d["STEPS"]
    KA, RPAD, RB, S, TCORE = d["KA"], d["RPAD"], d["RB"], d["S"], d["TCORE"]
    dt = mybir.dt
    AF = mybir.ActivationFunctionType
    ALU = mybir.AluOpType
    AX = mybir.AxisListType

    nc = bacc.Bacc(None, target_bir_lowering=False, num_devices=NCORES)

    XT = nc.dram_tensor("xT", [d["KAUG"], RPAD], dt.float32, kind="ExternalInput")
    WIH = nc.dram_tensor("Wih", [d["KAUG"], H4], dt.float32, kind="ExternalInput")
    WHH = nc.dram_tensor("Whh", [HH, H4], dt.float16, kind="ExternalInput")
    WLH = nc.dram_tensor("WlinHi", [HH, TAGS], dt.float16, kind="ExternalInput")
    WLL = nc.dram_tensor("WlinLo", [HH, TAGS], dt.float16, kind="ExternalInput")
    BLIN = nc.dram_tensor("blin", [16, 1], dt.float32, kind="ExternalInput")
    TRT = nc.dram_tensor("transTflat", [1, 100], dt.float32, kind="ExternalInput")
    STV = nc.dram_tensor("stopv", [1, TAGS], dt.float32, kind="ExternalInput")
    IDF = nc.dram_tensor("identflat", [1, 100], dt.float32, kind="ExternalInput")
    IDB = nc.dram_tensor("identB", [cfg["B"], cfg["B"]], dt.float16, kind="ExternalInput")
    SCORE = nc.dram_tensor("score", [1, 1], dt.float32, kind="ExternalOutput")
    BPATH = nc.dram_tensor("best_path", [1, T], dt.int32, kind="ExternalOutput")
    DBGF = nc.dram_tensor("dbg_feats", [TAGS, T], dt.float32, kind="ExternalOutput")

    with tile.TileContext(nc) as tc:
        with tc.tile_pool(name="dram", bufs=1, space="DRAM") as dpool:
            xp_d = dpool.tile([L, RPAD // L, H4], dt.float32, tag="xp_d")
            ag_in = dpool.tile([16, TCORE], dt.float32, tag="ag_in")
            ag_out = dpool.tile([16 * NCORES, TCORE], dt.float32,
                                addr_space="Shared", tag="ag_out")
            ftT_d = dpool.tile([TAGS, T], dt.float32, tag="ftT_d")
            sp_d = dpool.tile([128, 100], dt.float32, tag="sp_d")
            ss_d = dpool.tile([128, 1], dt.float32, tag="ss_d")
            g16_d = dpool.tile([16, 100], dt.float32, tag="g16_d")
            gs16_d = dpool.tile([16, 1], dt.float32, tag="gs16_d")
            e16_d = dpool.tile([16, 100], dt.float32, tag="e16_d")
            es16_d = dpool.tile([16, 1], dt.float32, tag="es16_d")
            grid_d = dpool.tile([128, 100], dt.float32, tag="grid_d")
            grs_d = dpool.tile([128, 1], dt.float32, tag="grs_d")

            # ================= Phase A: xp GEMM =================
            with (
                nc.named_scope("phA"),
                tc.tile_pool(name="sbA", bufs=1) as sba,
                tc.tile_pool(name="wpan", bufs=2) as wpan,
                tc.tile_pool(name="psA", bufs=4, space="PSUM") as psa,
                tc.tile_pool(name="outA", bufs=4) as outa,
            ):
                xp_writes = []
                xt = sba.tile([128, KA, RPAD], dt.float32r, tag="xt")
                nc.sync.dma_start(
                    xt[:],
                    XT[:].rearrange("(k p) r -> p k r", p=128).bitcast(dt.float32r))
                for j in range(NCH):
                    wp = wpan.tile([128, KA, 512], dt.float32r, tag="wp")
                    nc.sync.dma_start(
                        wp[:],
                        WIH[:].rearrange("(k p) n -> p k n", p=128)
                        [:, :, 512 * j:512 * (j + 1)].bitcast(dt.float32r))
                    for rb in range(RB):
                        pj = psa.tile([128, 512], dt.float32, tag="pj")
                        for k in range(KA):
                            nc.tensor.matmul(pj[:], xt[:, k, 128 * rb:128 * (rb + 1)],
                                             wp[:, k, :], start=(k == 0),
                                             stop=(k == KA - 1))
                        ot = outa.tile([128, 512], dt.float32, tag="ot")
                        nc.vector.tensor_copy(ot[:], pj[:])
                        PPER = 128 // L
                        wi = nc.sync.dma_start(
                            xp_d[:, PPER * rb:PPER * (rb + 1), 512 * j:512 * (j + 1)]
                            .rearrange("m j c -> j m c"), ot[:])
                        xp_writes.append(wi.ins)

            # ================= Phase B + C =================
            with nc.named_scope("phB"), tc.tile_pool(name="sbB", bufs=1) as sbb:
                whh = sbb.tile([128, KH, H4], dt.float16, tag="whh")
                nc.sync.dma_start(whh[:], WHH[:].rearrange("(k p) n -> p k n", p=128))
                hist = [sbb.tile([128, TCORE], dt.float16, tag=f"hist{k}", name=f"hist{k}")
                        for k in range(KH)]
                halo = [sbb.tile([128, 2 * B], dt.float16, tag=f"halo{k}", name=f"halo{k}")
                        for k in range(KH)]
                zer = sbb.tile([128, B], dt.float16, tag="zer")
                nc.vector.memset(zer[:], 0.0)
                cst = sbb.tile([B, HH], dt.float32, tag="cst")
                nc.vector.memset(cst[:], 0.0)
                sgi = sbb.tile([B, HH], dt.float32, tag="sgi")
                sgf = sbb.tile([B, HH], dt.float32, tag="sgf")
                sgg = sbb.tile([B, HH], dt.float32, tag="sgg")
                sgo = sbb.tile([B, HH], dt.float32, tag="sgo")
                tnc = sbb.tile([B, HH], dt.float32, tag="tnc")
                tm1 = sbb.tile([B, HH], dt.float32, tag="tm1")
                tm2 = sbb.tile([B, HH], dt.float32, tag="tm2")
                h16 = sbb.tile([B, HH], dt.float16, tag="h16")
                gsb = [sbb.tile([B, 512], dt.float32, tag=f"gsb{n}", name=f"gsb{n}") for n in range(NCH)]
                sgate = {0: sgi, 1: sgf, 2: sgg, 3: sgo}

                def hsrc(t, k):
                    if t < 0:
                        return zer[:]
                    if t < W:
                        return halo[k][:, (t % 2) * B:(t % 2 + 1) * B]
                    return hist[k][:, (t - W) * B:(t - W + 1) * B]

                with (
                    tc.tile_pool(name="xqp", bufs=2) as xqp,
                    tc.tile_pool(name="psB", bufs=1, space="PSUM") as psb,
                ):
                    import os as _os2
                    NPG = NCH if _os2.environ.get("NPG_FULL") else min(4, NCH)
                    pg = [psb.tile([B, 512], dt.float32, tag=f"pg{n}", name=f"pg{n}")
                          for n in range(NPG)]
                    ptx = ([] if _os2.environ.get("NPG_FULL") else
                           [psb.tile([128, 4 * B], dt.float16, tag=f"ptx{i}", name=f"ptx{i}")
                            for i in range(2)])
                    idb = sbb.tile([B, B], dt.float16, tag="idb")
                    nc.sync.dma_start(idb[:], IDB[:])
                    from concourse.tile_rust import add_dep_helper as _adh
                    fence_t = sbb.tile([1, 1], dt.float32, tag="fence_t")
                    fi = nc.vector.memset(fence_t[:], 0.0)
                    for wi_ in xp_writes:
                        _adh(fi.ins, wi_, sync=True, reason="xp fence in")
                    for t in range(STEPS):
                        xq = xqp.tile([B, H4], dt.float32, tag="xq")
                        ri = nc.sync.dma_start(xq[:], xp_d[t % L, t // L:t // L + B, :])
                        _adh(ri.ins, fi.ins, sync=True, reason="xp fence out")
                        for n in range(NCH):
                            for k in range(KH):
                                nc.tensor.matmul(pg[n % NPG][:], hsrc(t - 1, k),
                                                 whh[:, k, 512 * n:512 * (n + 1)],
                                                 start=(k == 0), stop=(k == KH - 1))
                            nc.vector.tensor_tensor(gsb[n][:], pg[n % NPG][:],
                                                    xq[:, 512 * n:512 * (n + 1)],
                                                    ALU.add)
                            c0, c1 = 512 * n, 512 * (n + 1)
                            for g in range(c0 // HH, (c1 - 1) // HH + 1):
                                lo, hi = max(c0, g * HH), min(c1, (g + 1) * HH)
                                fn = AF.Tanh if g == 2 else AF.Sigmoid
                                nc.scalar.activation(
                                    sgate[g][:, lo - g * HH:hi - g * HH],
                                    gsb[n][:, lo - c0:hi - c0], fn)
                        nc.vector.tensor_tensor(tm1[:], sgf[:], cst[:], ALU.mult)
                        nc.vector.tensor_tensor(tm2[:], sgi[:], sgg[:], ALU.mult)
                        nc.vector.tensor_tensor(cst[:], tm1[:], tm2[:], ALU.add)
                        nc.scalar.activation(tnc[:], cst[:], AF.Tanh)
                        nc.vector.tensor_tensor(h16[:], sgo[:], tnc[:], ALU.mult)
                        import os as _os
                        for k in range(KH):
                            dst = (halo[k][:, (t % 2) * B:(t % 2 + 1) * B] if t < W
                                   else hist[k][:, (t - W) * B:(t - W + 1) * B])
                            if _os.environ.get("NO_PE_T"):
                                nc.sync.dma_start_transpose(
                                    dst, h16[:, 128 * k:128 * (k + 1)])
                            else:
                                pslot = ptx[(k // 4) % 2][:, (k % 4) * B:(k % 4 + 1) * B]
                                nc.tensor.transpose(pslot,
                                                    h16[:, 128 * k:128 * (k + 1)],
                                                    idb[:])
                                nc.vector.tensor_copy(dst, pslot)

                # ---- Phase C: partial feats ----
                with (
                    nc.named_scope("phC"),
                    tc.tile_pool(name="sbC", bufs=1) as sbc,
                    tc.tile_pool(name="psC", bufs=2, space="PSUM") as psc,
                ):
                    wlh = sbc.tile([128, KH, TAGS], dt.float16, tag="wlh")
                    wll = sbc.tile([128, KH, TAGS], dt.float16, tag="wll")
                    nc.sync.dma_start(wlh[:], WLH[:].rearrange("(k p) n -> p k n", p=128))
                    nc.sync.dma_start(wll[:], WLL[:].rearrange("(k p) n -> p k n", p=128))
                    fpart = sbc.tile([16, TCORE], dt.float32, tag="fpart")
                    nc.vector.memset(fpart[:], 0.0)
                    FW = min(512, TCORE)
                    for n2 in range(TCORE // FW):
                        pf = psc.tile([TAGS, FW], dt.float32, tag="pf")
                        first = True
                        for k in range(KH):
                            for wl in (wlh, wll):
                                nc.tensor.matmul(
                                    pf[:], wl[:, k, :],
                                    hist[k][:, FW * n2:FW * (n2 + 1)],
                                    start=first,
                                    stop=(k == KH - 1 and wl is wll))
                                first = False
                        nc.vector.tensor_copy(
                            fpart[0:TAGS, FW * n2:FW * (n2 + 1)], pf[:])
                    nc.sync.dma_start(ag_in[:], fpart[:])

            nc.gpsimd.collective_compute(
                "AllGather", mybir.AluOpType.bypass,
                replica_groups=[list(range(NCORES))],
                ins=[ag_in[:].opt()], outs=[ag_out[:].opt()])

            # ================= Phase D: assemble feats =================
            with nc.named_scope("phD"), tc.tile_pool(name="sbD", bufs=1) as sbd:
                ag_c = [sbd.tile([16, TCORE], dt.float32, tag=f"ag{c}", name=f"ag{c}")
                        for c in range(NCORES)]
                for c in range(NCORES):
                    nc.sync.dma_start(ag_c[c][:], ag_out[16 * c:16 * (c + 1), :])
                blin_sb = sbd.tile([16, 1], dt.float32, tag="blin_sb")
                nc.sync.dma_start(blin_sb[:], BLIN[:])
                ftT = sbd.tile([TAGS, T], dt.float32, tag="ftT")
                for q in range(4):
                    fwd = (ag_c[q][0:TAGS, :]
                           .rearrange("p (t b) -> p b t", b=B))
                    bwd = (ag_c[7 - q][0:TAGS, ::-1]
                           .rearrange("p (t b) -> p b t", b=B))
                    nc.vector.tensor_tensor(
                        ftT[:, TCORE * q:TCORE * (q + 1)]
                        .rearrange("p (b t) -> p b t", b=B),
                        fwd, bwd, ALU.add)
                nc.vector.tensor_scalar_add(ftT[:], ftT[:], blin_sb[0:TAGS, 0:1])
                nc.sync.dma_start(DBGF[:], ftT[:])
                nc.sync.dma_start(ftT_d[:], ftT[:])

            # ================= Phase E: Viterbi =================
            with nc.named_scope("phE"), tc.tile_pool(name="sbE", bufs=1) as sbe:
                trt = sbe.tile([128, 100], dt.float32, tag="trt")
                stv = sbe.tile([128, TAGS], dt.float32, tag="stv")
                idf = sbe.tile([16, 100], dt.float32, tag="idf")
                zro = sbe.tile([16, 1], dt.float32, tag="zro")
                nc.vector.memset(zro[:], 0.0)
                nc.sync.dma_start(trt[0:1, :], TRT[:])
                nc.sync.dma_start(stv[0:1, :], STV[:])
                nc.sync.dma_start(idf[0:1, :], IDF[:])
                p = 1
                while p < 128:
                    q = min(p, 128 - p)
                    nc.sync.dma_start(trt[p:p + q, :], trt[0:q, :])
                    nc.sync.dma_start(stv[p:p + q, :], stv[0:q, :])
                    if p < 16:
                        q2 = min(p, 16 - p)
                        nc.sync.dma_start(idf[p:p + q2, :], idf[0:q2, :])
                    p *= 2

                ftseg = sbe.tile([128, TAGS, S], dt.float32, tag="ftseg")
                nc.sync.dma_start(ftseg[:],
                                  ftT_d[:].rearrange("n (q r) -> q n r", r=S))
                leaf = sbe.tile([128, S * 100], dt.float32, tag="leaf")
                nc.vector.tensor_tensor(
                    leaf[:].rearrange("q (r i n) -> q r i n", i=TAGS, n=TAGS),
                    trt[:].rearrange("q (i n) -> q i n", i=TAGS)
                    .unsqueeze(1).broadcast_to([128, S, TAGS, TAGS]),
                    ftseg[:].rearrange("q n r -> q r n")
                    .unsqueeze(2).broadcast_to([128, S, TAGS, TAGS]),
                    ALU.add)

                csc = sbe.tile([128, 1000], dt.float32, tag="csc")
                mx1 = sbe.tile([128, 1], dt.float32, tag="mx1")

                def compose(av, bv, ov, shin, shout, P=128):
                    nc.vector.tensor_tensor(
                        csc[0:P, :].rearrange("q (i n k) -> q i n k",
                                              i=TAGS, n=TAGS),
                        av.rearrange("q (i k) -> q i k", i=TAGS)
                        .unsqueeze(2).broadcast_to([P, TAGS, TAGS, TAGS]),
                        bv.rearrange("q (k n) -> q n k", k=TAGS)
                        .unsqueeze(1).broadcast_to([P, TAGS, TAGS, TAGS]),
                        ALU.add)
                    nc.vector.tensor_reduce(
                        ov, csc[0:P, :].rearrange("q (in k) -> q in k", k=TAGS),
                        AX.X, ALU.max)
                    nc.vector.tensor_reduce(mx1[0:P, :], ov, AX.X, ALU.max)
                    nc.vector.tensor_scalar_sub(ov, ov, mx1[0:P, 0:1])
                    if len(shin) > 1:
                        nc.vector.tensor_tensor(shout, shin[0], shin[1], ALU.add)
                        nc.vector.tensor_tensor(shout, shout, mx1[0:P, :], ALU.add)
                    else:
                        nc.vector.tensor_tensor(shout, shin[0], mx1[0:P, :], ALU.add)

                def seg_scan(direction, tag):
                    pref = sbe.tile([128, S * 100], dt.float32, tag="pref" + tag)
                    psh = sbe.tile([128, S], dt.float32, tag="psh" + tag)
                    nc.vector.memset(psh[:], 0.0)
                    rng = list(range(S)) if direction > 0 else list(range(S - 1, -1, -1))
                    r0 = rng[0]
                    nc.vector.tensor_copy(pref[:, r0 * 100:(r0 + 1) * 100],
                                          leaf[:, r0 * 100:(r0 + 1) * 100])
                    for r in rng[1:]:
                        prev = r - direction
                        lv = leaf[:, r * 100:(r + 1) * 100]
                        pv = pref[:, prev * 100:(prev + 1) * 100]
                        av, bv = (pv, lv) if direction > 0 else (lv, pv)
                        compose(av, bv, pref[:, r * 100:(r + 1) * 100],
                                [psh[:, prev:prev + 1]], psh[:, r:r + 1])
                    return pref, psh

                def cross_scan(pref, psh, direction, tag):
                    """exclusive scan over the 128 segment products."""
                    last = S - 1 if direction > 0 else 0
                    nc.sync.dma_start(sp_d[:], pref[:, last * 100:(last + 1) * 100])
                    nc.sync.dma_start(ss_d[:], psh[:, last:last + 1])
                    l1 = sbe.tile([16, 800], dt.float32, tag="l1" + tag)
                    l1s = sbe.tile([16, 8], dt.float32, tag="l1s" + tag)
                    nc.sync.dma_start(l1[:], sp_d[:].rearrange("(g j) e -> g (j e)", j=8))
                    nc.sync.dma_start(l1s[:], ss_d[:].rearrange("(g j) e -> g (j e)", j=8))
                    rng = list(range(8)) if direction > 0 else list(range(7, -1, -1))
                    for j in rng[1:]:
                        prev = j - direction
                        sv = l1[:, j * 100:(j + 1) * 100]
                        pv = l1[:, prev * 100:(prev + 1) * 100]
                        av, bv = (pv, sv) if direction > 0 else (sv, pv)
                        compose(av, bv, sv, [l1s[:, prev:prev + 1], l1s[:, j:j + 1]],
                                l1s[:, j:j + 1], P=16)
                    lastj = 7 if direction > 0 else 0
                    nc.sync.dma_start(g16_d[:], l1[:, lastj * 100:(lastj + 1) * 100])
                    nc.sync.dma_start(gs16_d[:], l1s[:, lastj:lastj + 1])
                    l2 = sbe.tile([1, 1600], dt.float32, tag="l2" + tag)
                    l2s = sbe.tile([1, 16], dt.float32, tag="l2s" + tag)
                    nc.sync.dma_start(l2[0:1, :],
                                      g16_d[:].rearrange("g e -> (g e)").unsqueeze(0))
                    nc.sync.dma_start(l2s[0:1, :],
                                      gs16_d[:].rearrange("g e -> (g e)").unsqueeze(0))
                    rng2 = list(range(16)) if direction > 0 else list(range(15, -1, -1))
                    for j in rng2[1:]:
                        prev = j - direction
                        sv = l2[:, j * 100:(j + 1) * 100]
                        pv = l2[:, prev * 100:(prev + 1) * 100]
                        av, bv = (pv, sv) if direction > 0 else (sv, pv)
                        compose(av, bv, sv, [l2s[:, prev:prev + 1], l2s[:, j:j + 1]],
                                l2s[:, j:j + 1], P=1)
                    # exclusive level-2 (group) prefixes -> e16_d (+ shifts)
                    if direction > 0:
                        nc.sync.dma_start(e16_d[1:16, :], l2[0:1, 0:1500])
                        nc.sync.dma_start(es16_d[1:16, :], l2s[0:1, 0:15])
                        nc.sync.dma_start(e16_d[0:1, :], idf[0:1, :])
                        nc.sync.dma_start(es16_d[0:1, :], zro[0:1, :])
                    else:
                        nc.sync.dma_start(e16_d[0:15, :], l2[0:1, 100:1600])
                        nc.sync.dma_start(es16_d[0:15, :], l2s[0:1, 1:16])
                        nc.sync.dma_start(e16_d[15:16, :], idf[0:1, :])
                        nc.sync.dma_start(es16_d[15:16, :], zro[0:1, :])
                    # shifted within-group prefixes -> grid_d (+ shifts)
                    gv = grid_d[:].rearrange("(h s) e -> h s e", s=8)
                    gsv = grs_d[:].rearrange("(h s) e -> h s e", s=8)
                    if direction > 0:
                        nc.sync.dma_start(gv[:, 1:8, :], l1[:, 0:700])
                        nc.sync.dma_start(gsv[:, 1:8, :], l1s[:, 0:7])
                        nc.sync.dma_start(gv[:, 0:1, :], idf[:, :].unsqueeze(1))
                        nc.sync.dma_start(gsv[:, 0:1, :], zro[:, :].unsqueeze(1))
                    else:
                        nc.sync.dma_start(gv[:, 0:7, :], l1[:, 100:800])
                        nc.sync.dma_start(gsv[:, 0:7, :], l1s[:, 1:8])
                        nc.sync.dma_start(gv[:, 7:8, :], idf[:, :].unsqueeze(1))
                        nc.sync.dma_start(gsv[:, 7:8, :], zro[:, :].unsqueeze(1))
                    # materialize exc [128, 100]
                    arow = sbe.tile([128, 100], dt.float32, tag="arow" + tag)
                    ars = sbe.tile([128, 1], dt.float32, tag="ars" + tag)
                    nc.sync.dma_start(
                        arow[:], e16_d[:].unsqueeze(1).broadcast_to([16, 8, 100]))
                    nc.sync.dma_start(
                        ars[:], es16_d[:].unsqueeze(1).broadcast_to([16, 8, 1]))
                    brow = sbe.tile([128, 100], dt.float32, tag="brow" + tag)
                    brs = sbe.tile([128, 1], dt.float32, tag="brs" + tag)
                    nc.sync.dma_start(brow[:], grid_d[:])
                    nc.sync.dma_start(brs[:], grs_d[:])
                    exc = sbe.tile([128, 100], dt.float32, tag="exc" + tag)
                    excs = sbe.tile([128, 1], dt.float32, tag="excs" + tag)
                    av, bv = (arow[:], brow[:]) if direction > 0 else (brow[:], arow[:])
                    compose(av, bv, exc[:], [ars[:], brs[:]], excs[:])
                    return exc, excs

                prefP, pshP = seg_scan(+1, "P")
                excP, excPs = cross_scan(prefP, pshP, +1, "P")
                prefS, pshS = seg_scan(-1, "S")
                excS, excSs = cross_scan(prefS, pshS, -1, "S")

                # ---- alpha: a[t, n] = max_k excP[q][START, k] + prefP[q,r][k, n]
                cbig = sbe.tile([128, S * 100], dt.float32, tag="cbig")
                aseg = sbe.tile([128, S * TAGS], dt.float32, tag="aseg")
                nc.vector.tensor_tensor(
                    cbig[:].rearrange("q (r n k) -> q r n k", n=TAGS, k=TAGS),
                    excP[:, START * TAGS:(START + 1) * TAGS]
                    .unsqueeze(1).unsqueeze(2).broadcast_to([128, S, TAGS, TAGS]),
                    prefP[:].rearrange("q (r k n) -> q r n k", k=TAGS, n=TAGS),
                    ALU.add)
                nc.vector.tensor_reduce(
                    aseg[:], cbig[:].rearrange("q (rn k) -> q rn k", k=TAGS),
                    AX.X, ALU.max)

                # ---- z[q][m] = max_k excS[q][m, k] + stopv[k]
                zq = sbe.tile([128, TAGS], dt.float32, tag="zq")
                nc.vector.tensor_tensor(
                    csc[:, 0:100].rearrange("q (m k) -> q m k", m=TAGS),
                    stv[:].unsqueeze(1).broadcast_to([128, TAGS, TAGS]),
                    excS[:].rearrange("q (m k) -> q m k", m=TAGS),
                    ALU.add)
                nc.vector.tensor_reduce(
                    zq[:], csc[:, 0:100].rearrange("q (m k) -> q m k", m=TAGS),
                    AX.X, ALU.max)

                # ---- beta: b[t, i] = max_m prefS[q, r+1][i, m] + z[q][m]
                bseg = sbe.tile([128, S * TAGS], dt.float32, tag="bseg")
                nc.vector.tensor_tensor(
                    cbig[:, 0:(S - 1) * 100]
                    .rearrange("q (r i m) -> q r i m", i=TAGS, m=TAGS),
                    prefS[:, 100:].rearrange("q (r i m) -> q r i m", i=TAGS, m=TAGS),
                    zq[:].unsqueeze(1).unsqueeze(2)
                    .broadcast_to([128, S - 1, TAGS, TAGS]),
                    ALU.add)
                nc.vector.tensor_reduce(
                    bseg[:, 0:(S - 1) * TAGS],
                    cbig[:, 0:(S - 1) * 100].rearrange("q (ri m) -> q ri m", m=TAGS),
                    AX.X, ALU.max)
                nc.vector.tensor_copy(bseg[:, (S - 1) * TAGS:S * TAGS], zq[:])

                # ---- path = argmax(alpha + beta) ----
                sc = sbe.tile([128, S * TAGS], dt.float32, tag="sc")
                nc.vector.tensor_tensor(sc[:], aseg[:], bseg[:], ALU.add)
                mv8 = sbe.tile([128, 8], dt.float32, tag="mv8")
                mi8 = sbe.tile([128, 8], dt.uint32, tag="mi8")
                path = sbe.tile([128, S], dt.int32, tag="path")
                for r in range(S):
                    nc.vector.max_with_indices(mv8[:], mi8[:],
                                               sc[:, r * TAGS:(r + 1) * TAGS])
                    nc.vector.tensor_copy(path[:, r:r + 1],
                                          mi8[:, 0:1].bitcast(dt.int32))
                nc.sync.dma_start(
                    BPATH[0:1, :].rearrange("p (q r) -> (p q) r", q=128), path[:])

                # ---- score = max(alpha_{T-1} + stopv) + shifts ----
                ts0 = sbe.tile([1, TAGS], dt.float32, tag="ts0")
                tsh = sbe.tile([1, 2], dt.float32, tag="tsh")
                nc.sync.dma_start(ts0[:], aseg[127:128, (S - 1) * TAGS:S * TAGS])
                nc.sync.dma_start(tsh[:, 0:1], excPs[127:128, :])
                nc.sync.dma_start(tsh[:, 1:2], pshP[127:128, S - 1:S])
                ts1 = sbe.tile([1, TAGS], dt.float32, tag="ts1")
                ts2 = sbe.tile([1, 1], dt.float32, tag="ts2")
                nc.vector.tensor_tensor(ts1[:], ts0[:], stv[0:1, :], ALU.add)
                nc.vector.tensor_reduce(ts2[:], ts1[:], AX.X, ALU.max)
                nc.vector.tensor_tensor(ts2[:], ts2[:], tsh[:, 0:1], ALU.add)
                nc.vector.tensor_tensor(ts2[:], ts2[:], tsh[:, 1:2], ALU.add)
                nc.sync.dma_start(SCORE[:], ts2[:])
    nc.compile()
    return nc


# ---------------------------------------------------------------------------
# host side
# ---------------------------------------------------------------------------

def prep_inputs(cfg, sentence, W_ih_f, W_hh_f, b_f, W_ih_b, W_hh_b, b_b,
                W_lin, b_lin, transitions):
    d = _derive(cfg)
    T, F, HH, W = cfg["T"], cfg["F"], cfg["HH"], cfg["W"]
    x = np.ascontiguousarray(sentence[:, 0, :], dtype=np.float32)

    def f16(a):
        return a.astype(np.float16)

    def core_inputs(seq, W_ih, W_hh, b, wl_half):
        outs = []
        waug = np.zeros((d["KAUG"], d["H4"]), np.float32)
        waug[0:F] = W_ih.T.astype(np.float32)
        waug[F] = b.astype(np.float32)
        wlh = f16(wl_half)
        wll = f16(wl_half - wlh.astype(np.float32))
        for q in range(4):
            s0 = q * d["TCORE"] - W
            rows = np.zeros((d["RPAD"], d["KAUG"]), np.float32)
            lo = max(s0, 0)
            rows[lo - s0:d["RROWS"], 0:F] = seq[lo:s0 + d["RROWS"]]
            rows[lo - s0:d["RROWS"], F] = 1.0
            outs.append({
                "xT": np.ascontiguousarray(rows.T),
                "Wih": waug,
                "Whh": np.ascontiguousarray(f16(W_hh.T)),
                "WlinHi": np.ascontiguousarray(wlh),
                "WlinLo": np.ascontiguousarray(wll),
            })
        return outs

    wlf = W_lin[:, 0:HH].T.astype(np.float32)      # [HH, 10]
    wlb = W_lin[:, HH:].T.astype(np.float32)
    cores = (core_inputs(x, W_ih_f, W_hh_f, b_f, wlf)
             + core_inputs(x[::-1].copy(), W_ih_b, W_hh_b, b_b, wlb))

    blin = np.zeros((16, 1), np.float32)
    blin[0:TAGS, 0] = b_lin
    trt = np.ascontiguousarray(
        transitions.T.astype(np.float32).reshape(1, 100))
    stopv = transitions[STOP, :].astype(np.float32).reshape(1, TAGS)
    idf = np.full((TAGS, TAGS), NEG, np.float32)
    np.fill_diagonal(idf, 0.0)
    idf = idf.reshape(1, 100)
    idb = np.eye(cfg["B"], dtype=np.float16)
    for m in cores:
        m["blin"] = blin
        m["transTflat"] = trt
        m["stopv"] = np.ascontiguousarray(stopv)
        m["identflat"] = idf
        m["identB"] = idb
    return cores


def kernel(**inputs):
    from concourse.bass_utils import run_bass_kernel_spmd
    cfg = CFG
    in_maps = prep_inputs(cfg, **{k: np.asarray(v) for k, v in inputs.items()})
    nc = build_nc(cfg)
    res = run_bass_kernel_spmd(nc, in_maps, list(range(NCORES)))
    r0 = res.results[0]
    score = np.float32(r0["score"][0, 0])
    path = r0["best_path"].reshape(-1).astype(np.int32)
    return score, path


# revision 19
# speedup vs baseline: 1.0252x; 1.0235x over previous
"""BiLSTM + CRF Viterbi decode on 8 trn2 NeuronCores (Bass/Tile, SPMD).

Strategy:
  - cores 0-3: forward LSTM over sequence quarters; cores 4-7: backward LSTM
    run as a forward LSTM over the host-reversed sequence. One SPMD program.
  - the serial recurrence is broken with chunked restart: each core runs
    B chunks of length L as a batch with a W-step warmup halo (forget-gate
    contraction makes the halo error ~1e-15 at W=64).
  - input projection xp = [X|mask] @ [W_ih.T; b] in fp32r, staged in DRAM;
    recurrence matmuls in fp16 (stationary h^T via DMA-transpose, moving
    W_hh^T, fp32 PSUM); gate math fp32 on ACT/DVE.
  - feats = h @ W_lin.T with fp16 hi/lo weights; partial feats AllGather'd
    and assembled on every core.
  - Viterbi via normalized max-plus segmented scan (prefix + suffix);
    path[t] = argmax(alpha_t + beta_t), replicated on every core.
"""

import numpy as np

CFG = dict(T=4096, F=2048, HH=1024, B=64, L=16, W=64)
TAGS, START, STOP = 10, 8, 9
NCORES = 8
NEG = -1e9


def _derive(cfg):
    d = dict(cfg)
    d["H4"] = 4 * cfg["HH"]
    d["KH"] = cfg["HH"] // 128
    d["NCH"] = d["H4"] // 512
    d["STEPS"] = cfg["W"] + cfg["L"]
    d["TCORE"] = cfg["T"] // 4
    assert cfg["B"] * cfg["L"] == d["TCORE"]
    d["KAUG"] = cfg["F"] + 128
    d["KA"] = d["KAUG"] // 128
    d["RROWS"] = d["TCORE"] + cfg["W"]
    d["RPAD"] = ((d["RROWS"] + 127) // 128) * 128
    d["RB"] = d["RPAD"] // 128
    d["S"] = cfg["T"] // 128
    return d


# ---------------------------------------------------------------------------
# device program
# ---------------------------------------------------------------------------

def build_nc(cfg):
    import concourse.bacc as bacc
    import concourse.mybir as mybir
    import concourse.tile as tile

    d = _derive(cfg)
    T, F, HH, B, L, W = (cfg[k] for k in ("T", "F", "HH", "B", "L", "W"))
    H4, KH, NCH, STEPS = d["H4"], d["KH"], d["NCH"], d["STEPS"]
    KA, RPAD, RB, S, TCORE = d["KA"], d["RPAD"], d["RB"], d["S"], d["TCORE"]
    dt = mybir.dt
    AF = mybir.ActivationFunctionType
    ALU = mybir.AluOpType
    AX = mybir.AxisListType

    nc = bacc.Bacc(None, target_bir_lowering=False, num_devices=NCORES)

    XT = nc.dram_tensor("xT", [d["KAUG"], RPAD], dt.float32, kind="ExternalInput")
    WIH = nc.dram_tensor("Wih", [d["KAUG"], H4], dt.float32, kind="ExternalInput")
    WHH = nc.dram_tensor("Whh", [HH, H4], dt.float16, kind="ExternalInput")
    WLH = nc.dram_tensor("WlinHi", [HH, TAGS], dt.float16, kind="ExternalInput")
    WLL = nc.dram_tensor("WlinLo", [HH, TAGS], dt.float16, kind="ExternalInput")
    BLIN = nc.dram_tensor("blin", [16, 1], dt.float32, kind="ExternalInput")
    TRT = nc.dram_tensor("transTflat", [1, 100], dt.float32, kind="ExternalInput")
    STV = nc.dram_tensor("stopv", [1, TAGS], dt.float32, kind="ExternalInput")
    IDF = nc.dram_tensor("identflat", [1, 100], dt.float32, kind="ExternalInput")
    IDB = nc.dram_tensor("identB", [cfg["B"], cfg["B"]], dt.float16, kind="ExternalInput")
    SCORE = nc.dram_tensor("score", [1, 1], dt.float32, kind="ExternalOutput")
    BPATH = nc.dram_tensor("best_path", [1, T], dt.int32, kind="ExternalOutput")
    DBGF = nc.dram_tensor("dbg_feats", [TAGS, T], dt.float32, kind="ExternalOutput")

    with tile.TileContext(nc) as tc:
        with tc.tile_pool(name="dram", bufs=1, space="DRAM") as dpool:
            import os as _os0
            XP_FLAT = bool(_os0.environ.get("XP_FLAT"))
            xp_d = (dpool.tile([RPAD, H4], dt.float32, tag="xp_d", name="xp_d")
                    if XP_FLAT else
                    dpool.tile([L, RPAD // L, H4], dt.float32, tag="xp_d", name="xp_d"))
            ag_in = dpool.tile([16, TCORE], dt.float32, tag="ag_in")
            ag_out = dpool.tile([16 * NCORES, TCORE], dt.float32,
                                addr_space="Shared", tag="ag_out")
            ftT_d = dpool.tile([TAGS, T], dt.float32, tag="ftT_d")
            sp_d = dpool.tile([128, 100], dt.float32, tag="sp_d")
            ss_d = dpool.tile([128, 1], dt.float32, tag="ss_d")
            g16_d = dpool.tile([16, 100], dt.float32, tag="g16_d")
            gs16_d = dpool.tile([16, 1], dt.float32, tag="gs16_d")
            e16_d = dpool.tile([16, 100], dt.float32, tag="e16_d")
            es16_d = dpool.tile([16, 1], dt.float32, tag="es16_d")
            grid_d = dpool.tile([128, 100], dt.float32, tag="grid_d")
            grs_d = dpool.tile([128, 1], dt.float32, tag="grs_d")

            # ================= Phase A: xp GEMM =================
            with (
                nc.named_scope("phA"),
                tc.tile_pool(name="sbA", bufs=1) as sba,
                tc.tile_pool(name="wpan", bufs=2) as wpan,
                tc.tile_pool(name="psA", bufs=4, space="PSUM") as psa,
                tc.tile_pool(name="outA", bufs=4) as outa,
            ):
                xp_writes = []
                xt = sba.tile([128, KA, RPAD], dt.float32r, tag="xt")
                nc.sync.dma_start(
                    xt[:],
                    XT[:].rearrange("(k p) r -> p k r", p=128).bitcast(dt.float32r))
                for j in range(NCH):
                    wp = wpan.tile([128, KA, 512], dt.float32r, tag="wp")
                    nc.sync.dma_start(
                        wp[:],
                        WIH[:].rearrange("(k p) n -> p k n", p=128)
                        [:, :, 512 * j:512 * (j + 1)].bitcast(dt.float32r))
                    for rb in range(RB):
                        pj = psa.tile([128, 512], dt.float32, tag="pj")
                        for k in range(KA):
                            nc.tensor.matmul(pj[:], xt[:, k, 128 * rb:128 * (rb + 1)],
                                             wp[:, k, :], start=(k == 0),
                                             stop=(k == KA - 1))
                        ot = outa.tile([128, 512], dt.float32, tag="ot")
                        nc.vector.tensor_copy(ot[:], pj[:])
                        PPER = 128 // L
                        if XP_FLAT:
                            wi = nc.sync.dma_start(
                                xp_d[128 * rb:128 * (rb + 1), 512 * j:512 * (j + 1)], ot[:])
                        else:
                            wi = nc.sync.dma_start(
                                xp_d[:, PPER * rb:PPER * (rb + 1), 512 * j:512 * (j + 1)]
                                .rearrange("m j c -> j m c"), ot[:])
                        xp_writes.append(wi.ins)

            # ================= Phase B + C =================
            with nc.named_scope("phB"), tc.tile_pool(name="sbB", bufs=1) as sbb:
                whh = sbb.tile([128, KH, H4], dt.float16, tag="whh")
                nc.sync.dma_start(whh[:], WHH[:].rearrange("(k p) n -> p k n", p=128))
                hist = [sbb.tile([128, TCORE], dt.float16, tag=f"hist{k}", name=f"hist{k}")
                        for k in range(KH)]
                halo = [sbb.tile([128, 2 * B], dt.float16, tag=f"halo{k}", name=f"halo{k}")
                        for k in range(KH)]
                zer = sbb.tile([128, B], dt.float16, tag="zer")
                nc.vector.memset(zer[:], 0.0)
                cst = sbb.tile([B, HH], dt.float32, tag="cst")
                nc.vector.memset(cst[:], 0.0)
                sgi = sbb.tile([B, HH], dt.float32, tag="sgi")
                sgf = sbb.tile([B, HH], dt.float32, tag="sgf")
                sgg = sbb.tile([B, HH], dt.float32, tag="sgg")
                sgo = sbb.tile([B, HH], dt.float32, tag="sgo")
                tnc = sbb.tile([B, HH], dt.float32, tag="tnc")
                tm1 = sbb.tile([B, HH], dt.float32, tag="tm1")
                tm2 = sbb.tile([B, HH], dt.float32, tag="tm2")
                h16 = sbb.tile([B, HH], dt.float16, tag="h16")
                gsb = [sbb.tile([B, 512], dt.float32, tag=f"gsb{n}", name=f"gsb{n}") for n in range(NCH)]
                sgate = {0: sgi, 1: sgf, 2: sgg, 3: sgo}

                def hsrc(t, k):
                    if t < 0:
                        return zer[:]
                    if t < W:
                        return halo[k][:, (t % 2) * B:(t % 2 + 1) * B]
                    return hist[k][:, (t - W) * B:(t - W + 1) * B]

                with (
                    tc.tile_pool(name="xqp", bufs=2) as xqp,
                    tc.tile_pool(name="psB", bufs=1, space="PSUM") as psb,
                ):
                    import os as _os2
                    NPG = NCH if _os2.environ.get("NPG_FULL") else min(4, NCH)
                    pg = [psb.tile([B, 512], dt.float32, tag=f"pg{n}", name=f"pg{n}")
                          for n in range(NPG)]
                    ptx = ([] if _os2.environ.get("NPG_FULL") else
                           [psb.tile([128, 4 * B], dt.float16, tag=f"ptx{i}", name=f"ptx{i}")
                            for i in range(2)])
                    idb = sbb.tile([B, B], dt.float16, tag="idb")
                    nc.sync.dma_start(idb[:], IDB[:])
                    from concourse.tile_rust import add_dep_helper as _adh
                    fence_t = sbb.tile([1, 1], dt.float32, tag="fence_t")
                    fi = nc.vector.memset(fence_t[:], 0.0)
                    for wi_ in xp_writes:
                        _adh(fi.ins, wi_, sync=True, reason="xp fence in")
                    for t in range(STEPS):
                        xq = xqp.tile([B, H4], dt.float32, tag="xq")
                        ri = (nc.sync.dma_start(xq[:], xp_d[t:t + B * L:L, :])
                              if XP_FLAT else
                              nc.sync.dma_start(xq[:], xp_d[t % L, t // L:t // L + B, :]))
                        _adh(ri.ins, fi.ins, sync=True, reason="xp fence out")
                        for n in range(NCH):
                            for k in range(KH):
                                nc.tensor.matmul(pg[n % NPG][:], hsrc(t - 1, k),
                                                 whh[:, k, 512 * n:512 * (n + 1)],
                                                 start=(k == 0), stop=(k == KH - 1))
                            nc.vector.tensor_tensor(gsb[n][:], pg[n % NPG][:],
                                                    xq[:, 512 * n:512 * (n + 1)],
                                                    ALU.add)
                            c0, c1 = 512 * n, 512 * (n + 1)
                            for g in range(c0 // HH, (c1 - 1) // HH + 1):
                                lo, hi = max(c0, g * HH), min(c1, (g + 1) * HH)
                                fn = AF.Tanh if g == 2 else AF.Sigmoid
                                nc.scalar.activation(
                                    sgate[g][:, lo - g * HH:hi - g * HH],
                                    gsb[n][:, lo - c0:hi - c0], fn)
                        nc.vector.tensor_tensor(tm1[:], sgf[:], cst[:], ALU.mult)
                        nc.vector.tensor_tensor(tm2[:], sgi[:], sgg[:], ALU.mult)
                        nc.vector.tensor_tensor(cst[:], tm1[:], tm2[:], ALU.add)
                        nc.scalar.activation(tnc[:], cst[:], AF.Tanh)
                        nc.vector.tensor_tensor(h16[:], sgo[:], tnc[:], ALU.mult)
                        import os as _os
                        for k in range(KH):
                            dst = (halo[k][:, (t % 2) * B:(t % 2 + 1) * B] if t < W
                                   else hist[k][:, (t - W) * B:(t - W + 1) * B])
                            if _os.environ.get("NO_PE_T"):
                                nc.sync.dma_start_transpose(
                                    dst, h16[:, 128 * k:128 * (k + 1)])
                            else:
                                pslot = ptx[(k // 4) % 2][:, (k % 4) * B:(k % 4 + 1) * B]
                                nc.tensor.transpose(pslot,
                                                    h16[:, 128 * k:128 * (k + 1)],
                                                    idb[:])
                                nc.vector.tensor_copy(dst, pslot)

                # ---- Phase C: partial feats ----
                with (
                    nc.named_scope("phC"),
                    tc.tile_pool(name="sbC", bufs=1) as sbc,
                    tc.tile_pool(name="psC", bufs=2, space="PSUM") as psc,
                ):
                    wlh = sbc.tile([128, KH, TAGS], dt.float16, tag="wlh")
                    wll = sbc.tile([128, KH, TAGS], dt.float16, tag="wll")
                    nc.sync.dma_start(wlh[:], WLH[:].rearrange("(k p) n -> p k n", p=128))
                    nc.sync.dma_start(wll[:], WLL[:].rearrange("(k p) n -> p k n", p=128))
                    fpart = sbc.tile([16, TCORE], dt.float32, tag="fpart")
                    nc.vector.memset(fpart[:], 0.0)
                    FW = min(512, TCORE)
                    for n2 in range(TCORE // FW):
                        pf = psc.tile([TAGS, FW], dt.float32, tag="pf")
                        first = True
                        for k in range(KH):
                            for wl in (wlh, wll):
                                nc.tensor.matmul(
                                    pf[:], wl[:, k, :],
                                    hist[k][:, FW * n2:FW * (n2 + 1)],
                                    start=first,
                                    stop=(k == KH - 1 and wl is wll))
                                first = False
                        nc.vector.tensor_copy(
                            fpart[0:TAGS, FW * n2:FW * (n2 + 1)], pf[:])
                    nc.sync.dma_start(ag_in[:], fpart[:])

            nc.gpsimd.collective_compute(
                "AllGather", mybir.AluOpType.bypass,
                replica_groups=[list(range(NCORES))],
                ins=[ag_in[:].opt()], outs=[ag_out[:].opt()])

            # ================= Phase D: assemble feats =================
            with nc.named_scope("phD"), tc.tile_pool(name="sbD", bufs=1) as sbd:
                ag_c = [sbd.tile([16, TCORE], dt.float32, tag=f"ag{c}", name=f"ag{c}")
                        for c in range(NCORES)]
                for c in range(NCORES):
                    nc.sync.dma_start(ag_c[c][:], ag_out[16 * c:16 * (c + 1), :])
                blin_sb = sbd.tile([16, 1], dt.float32, tag="blin_sb")
                nc.sync.dma_start(blin_sb[:], BLIN[:])
                ftT = sbd.tile([TAGS, T], dt.float32, tag="ftT")
                for q in range(4):
                    fwd = (ag_c[q][0:TAGS, :]
                           .rearrange("p (t b) -> p b t", b=B))
                    bwd = (ag_c[7 - q][0:TAGS, ::-1]
                           .rearrange("p (t b) -> p b t", b=B))
                    nc.vector.tensor_tensor(
                        ftT[:, TCORE * q:TCORE * (q + 1)]
                        .rearrange("p (b t) -> p b t", b=B),
                        fwd, bwd, ALU.add)
                nc.vector.tensor_scalar_add(ftT[:], ftT[:], blin_sb[0:TAGS, 0:1])
                nc.sync.dma_start(DBGF[:], ftT[:])
                nc.sync.dma_start(ftT_d[:], ftT[:])

            # ================= Phase E: Viterbi =================
            with nc.named_scope("phE"), tc.tile_pool(name="sbE", bufs=1) as sbe:
                trt = sbe.tile([128, 100], dt.float32, tag="trt")
                stv = sbe.tile([128, TAGS], dt.float32, tag="stv")
                idf = sbe.tile([16, 100], dt.float32, tag="idf")
                zro = sbe.tile([16, 1], dt.float32, tag="zro")
                nc.vector.memset(zro[:], 0.0)
                nc.sync.dma_start(trt[0:1, :], TRT[:])
                nc.sync.dma_start(stv[0:1, :], STV[:])
                nc.sync.dma_start(idf[0:1, :], IDF[:])
                p = 1
                while p < 128:
                    q = min(p, 128 - p)
                    nc.sync.dma_start(trt[p:p + q, :], trt[0:q, :])
                    nc.sync.dma_start(stv[p:p + q, :], stv[0:q, :])
                    if p < 16:
                        q2 = min(p, 16 - p)
                        nc.sync.dma_start(idf[p:p + q2, :], idf[0:q2, :])
                    p *= 2

                ftseg = sbe.tile([128, TAGS, S], dt.float32, tag="ftseg")
                nc.sync.dma_start(ftseg[:],
                                  ftT_d[:].rearrange("n (q r) -> q n r", r=S))
                leaf = sbe.tile([128, S * 100], dt.float32, tag="leaf")
                nc.vector.tensor_tensor(
                    leaf[:].rearrange("q (r i n) -> q r i n", i=TAGS, n=TAGS),
                    trt[:].rearrange("q (i n) -> q i n", i=TAGS)
                    .unsqueeze(1).broadcast_to([128, S, TAGS, TAGS]),
                    ftseg[:].rearrange("q n r -> q r n")
                    .unsqueeze(2).broadcast_to([128, S, TAGS, TAGS]),
                    ALU.add)

                csc = sbe.tile([128, 1000], dt.float32, tag="csc")
                mx1 = sbe.tile([128, 1], dt.float32, tag="mx1")

                def compose(av, bv, ov, shin, shout, P=128):
                    nc.vector.tensor_tensor(
                        csc[0:P, :].rearrange("q (i n k) -> q i n k",
                                              i=TAGS, n=TAGS),
                        av.rearrange("q (i k) -> q i k", i=TAGS)
                        .unsqueeze(2).broadcast_to([P, TAGS, TAGS, TAGS]),
                        bv.rearrange("q (k n) -> q n k", k=TAGS)
                        .unsqueeze(1).broadcast_to([P, TAGS, TAGS, TAGS]),
                        ALU.add)
                    nc.vector.tensor_reduce(
                        ov, csc[0:P, :].rearrange("q (in k) -> q in k", k=TAGS),
                        AX.X, ALU.max)
                    nc.vector.tensor_reduce(mx1[0:P, :], ov, AX.X, ALU.max)
                    nc.vector.tensor_scalar_sub(ov, ov, mx1[0:P, 0:1])
                    if len(shin) > 1:
                        nc.vector.tensor_tensor(shout, shin[0], shin[1], ALU.add)
                        nc.vector.tensor_tensor(shout, shout, mx1[0:P, :], ALU.add)
                    else:
                        nc.vector.tensor_tensor(shout, shin[0], mx1[0:P, :], ALU.add)

                def seg_scan(direction, tag):
                    pref = sbe.tile([128, S * 100], dt.float32, tag="pref" + tag)
                    psh = sbe.tile([128, S], dt.float32, tag="psh" + tag)
                    nc.vector.memset(psh[:], 0.0)
                    rng = list(range(S)) if direction > 0 else list(range(S - 1, -1, -1))
                    r0 = rng[0]
                    nc.vector.tensor_copy(pref[:, r0 * 100:(r0 + 1) * 100],
                                          leaf[:, r0 * 100:(r0 + 1) * 100])
                    for r in rng[1:]:
                        prev = r - direction
                        lv = leaf[:, r * 100:(r + 1) * 100]
                        pv = pref[:, prev * 100:(prev + 1) * 100]
                        av, bv = (pv, lv) if direction > 0 else (lv, pv)
                        compose(av, bv, pref[:, r * 100:(r + 1) * 100],
                                [psh[:, prev:prev + 1]], psh[:, r:r + 1])
                    return pref, psh

                def cross_scan(pref, psh, direction, tag):
                    """exclusive scan over the 128 segment products."""
                    last = S - 1 if direction > 0 else 0
                    nc.sync.dma_start(sp_d[:], pref[:, last * 100:(last + 1) * 100])
                    nc.sync.dma_start(ss_d[:], psh[:, last:last + 1])
                    l1 = sbe.tile([16, 800], dt.float32, tag="l1" + tag)
                    l1s = sbe.tile([16, 8], dt.float32, tag="l1s" + tag)
                    nc.sync.dma_start(l1[:], sp_d[:].rearrange("(g j) e -> g (j e)", j=8))
                    nc.sync.dma_start(l1s[:], ss_d[:].rearrange("(g j) e -> g (j e)", j=8))
                    rng = list(range(8)) if direction > 0 else list(range(7, -1, -1))
                    for j in rng[1:]:
                        prev = j - direction
                        sv = l1[:, j * 100:(j + 1) * 100]
                        pv = l1[:, prev * 100:(prev + 1) * 100]
                        av, bv = (pv, sv) if direction > 0 else (sv, pv)
                        compose(av, bv, sv, [l1s[:, prev:prev + 1], l1s[:, j:j + 1]],
                                l1s[:, j:j + 1], P=16)
                    lastj = 7 if direction > 0 else 0
                    nc.sync.dma_start(g16_d[:], l1[:, lastj * 100:(lastj + 1) * 100])
                    nc.sync.dma_start(gs16_d[:], l1s[:, lastj:lastj + 1])
                    l2 = sbe.tile([1, 1600], dt.float32, tag="l2" + tag)
                    l2s = sbe.tile([1, 16], dt.float32, tag="l2s" + tag)
                    nc.sync.dma_start(l2[0:1, :],
                                      g16_d[:].rearrange("g e -> (g e)").unsqueeze(0))
                    nc.sync.dma_start(l2s[0:1, :],
                                      gs16_d[:].rearrange("g e -> (g e)").unsqueeze(0))
                    rng2 = list(range(16)) if direction > 0 else list(range(15, -1, -1))
                    for j in rng2[1:]:
                        prev = j - direction
                        sv = l2[:, j * 100:(j + 1) * 100]
                        pv = l2[:, prev * 100:(prev + 1) * 100]
                        av, bv = (pv, sv) if direction > 0 else (sv, pv)
                        compose(av, bv, sv, [l2s[:, prev:prev + 1], l2s[:, j:j + 1]],
                                l2s[:, j:j + 1], P=1)
                    # exclusive level-2 (group) prefixes -> e16_d (+ shifts)
                    if direction > 0:
                        nc.sync.dma_start(e16_d[1:16, :], l2[0:1, 0:1500])
                        nc.sync.dma_start(es16_d[1:16, :], l2s[0:1, 0:15])
                        nc.sync.dma_start(e16_d[0:1, :], idf[0:1, :])
                        nc.sync.dma_start(es16_d[0:1, :], zro[0:1, :])
                    else:
                        nc.sync.dma_start(e16_d[0:15, :], l2[0:1, 100:1600])
                        nc.sync.dma_start(es16_d[0:15, :], l2s[0:1, 1:16])
                        nc.sync.dma_start(e16_d[15:16, :], idf[0:1, :])
                        nc.sync.dma_start(es16_d[15:16, :], zro[0:1, :])
                    # shifted within-group prefixes -> grid_d (+ shifts)
                    gv = grid_d[:].rearrange("(h s) e -> h s e", s=8)
                    gsv = grs_d[:].rearrange("(h s) e -> h s e", s=8)
                    if direction > 0:
                        nc.sync.dma_start(gv[:, 1:8, :], l1[:, 0:700])
                        nc.sync.dma_start(gsv[:, 1:8, :], l1s[:, 0:7])
                        nc.sync.dma_start(gv[:, 0:1, :], idf[:, :].unsqueeze(1))
                        nc.sync.dma_start(gsv[:, 0:1, :], zro[:, :].unsqueeze(1))
                    else:
                        nc.sync.dma_start(gv[:, 0:7, :], l1[:, 100:800])
                        nc.sync.dma_start(gsv[:, 0:7, :], l1s[:, 1:8])
                        nc.sync.dma_start(gv[:, 7:8, :], idf[:, :].unsqueeze(1))
                        nc.sync.dma_start(gsv[:, 7:8, :], zro[:, :].unsqueeze(1))
                    # materialize exc [128, 100]
                    arow = sbe.tile([128, 100], dt.float32, tag="arow" + tag)
                    ars = sbe.tile([128, 1], dt.float32, tag="ars" + tag)
                    nc.sync.dma_start(
                        arow[:], e16_d[:].unsqueeze(1).broadcast_to([16, 8, 100]))
                    nc.sync.dma_start(
                        ars[:], es16_d[:].unsqueeze(1).broadcast_to([16, 8, 1]))
                    brow = sbe.tile([128, 100], dt.float32, tag="brow" + tag)
                    brs = sbe.tile([128, 1], dt.float32, tag="brs" + tag)
                    nc.sync.dma_start(brow[:], grid_d[:])
                    nc.sync.dma_start(brs[:], grs_d[:])
                    exc = sbe.tile([128, 100], dt.float32, tag="exc" + tag)
                    excs = sbe.tile([128, 1], dt.float32, tag="excs" + tag)
                    av, bv = (arow[:], brow[:]) if direction > 0 else (brow[:], arow[:])
                    compose(av, bv, exc[:], [ars[:], brs[:]], excs[:])
                    return exc, excs

                prefP, pshP = seg_scan(+1, "P")
                excP, excPs = cross_scan(prefP, pshP, +1, "P")
                prefS, pshS = seg_scan(-1, "S")
                excS, excSs = cross_scan(prefS, pshS, -1, "S")

                # ---- alpha: a[t, n] = max_k excP[q][START, k] + prefP[q,r][k, n]
                cbig = sbe.tile([128, S * 100], dt.float32, tag="cbig")
                aseg = sbe.tile([128, S * TAGS], dt.float32, tag="aseg")
                nc.vector.tensor_tensor(
                    cbig[:].rearrange("q (r n k) -> q r n k", n=TAGS, k=TAGS),
                    excP[:, START * TAGS:(START + 1) * TAGS]
                    .unsqueeze(1).unsqueeze(2).broadcast_to([128, S, TAGS, TAGS]),
                    prefP[:].rearrange("q (r k n) -> q r n k", k=TAGS, n=TAGS),
                    ALU.add)
                nc.vector.tensor_reduce(
                    aseg[:], cbig[:].rearrange("q (rn k) -> q rn k", k=TAGS),
                    AX.X, ALU.max)

                # ---- z[q][m] = max_k excS[q][m, k] + stopv[k]
                zq = sbe.tile([128, TAGS], dt.float32, tag="zq")
                nc.vector.tensor_tensor(
                    csc[:, 0:100].rearrange("q (m k) -> q m k", m=TAGS),
                    stv[:].unsqueeze(1).broadcast_to([128, TAGS, TAGS]),
                    excS[:].rearrange("q (m k) -> q m k", m=TAGS),
                    ALU.add)
                nc.vector.tensor_reduce(
                    zq[:], csc[:, 0:100].rearrange("q (m k) -> q m k", m=TAGS),
                    AX.X, ALU.max)

                # ---- beta: b[t, i] = max_m prefS[q, r+1][i, m] + z[q][m]
                bseg = sbe.tile([128, S * TAGS], dt.float32, tag="bseg")
                nc.vector.tensor_tensor(
                    cbig[:, 0:(S - 1) * 100]
                    .rearrange("q (r i m) -> q r i m", i=TAGS, m=TAGS),
                    prefS[:, 100:].rearrange("q (r i m) -> q r i m", i=TAGS, m=TAGS),
                    zq[:].unsqueeze(1).unsqueeze(2)
                    .broadcast_to([128, S - 1, TAGS, TAGS]),
                    ALU.add)
                nc.vector.tensor_reduce(
                    bseg[:, 0:(S - 1) * TAGS],
                    cbig[:, 0:(S - 1) * 100].rearrange("q (ri m) -> q ri m", m=TAGS),
                    AX.X, ALU.max)
                nc.vector.tensor_copy(bseg[:, (S - 1) * TAGS:S * TAGS], zq[:])

                # ---- path = argmax(alpha + beta) ----
                sc = sbe.tile([128, S * TAGS], dt.float32, tag="sc")
                nc.vector.tensor_tensor(sc[:], aseg[:], bseg[:], ALU.add)
                mv8 = sbe.tile([128, 8], dt.float32, tag="mv8")
                mi8 = sbe.tile([128, 8], dt.uint32, tag="mi8")
                path = sbe.tile([128, S], dt.int32, tag="path")
                for r in range(S):
                    nc.vector.max_with_indices(mv8[:], mi8[:],
                                               sc[:, r * TAGS:(r + 1) * TAGS])
                    nc.vector.tensor_copy(path[:, r:r + 1],
                                          mi8[:, 0:1].bitcast(dt.int32))
                nc.sync.dma_start(
                    BPATH[0:1, :].rearrange("p (q r) -> (p q) r", q=128), path[:])

                # ---- score = max(alpha_{T-1} + stopv) + shifts ----
                ts0 = sbe.tile([1, TAGS], dt.float32, tag="ts0")
                tsh = sbe.tile([1, 2], dt.float32, tag="tsh")
                nc.sync.dma_start(ts0[:], aseg[127:128, (S - 1) * TAGS:S * TAGS])
                nc.sync.dma_start(tsh[:, 0:1], excPs[127:128, :])
                nc.sync.dma_start(tsh[:, 1:2], pshP[127:128, S - 1:S])
                ts1 = sbe.tile([1, TAGS], dt.float32, tag="ts1")
                ts2 = sbe.tile([1, 1], dt.float32, tag="ts2")
                nc.vector.tensor_tensor(ts1[:], ts0[:], stv[0:1, :], ALU.add)
                nc.vector.tensor_reduce(ts2[:], ts1[:], AX.X, ALU.max)
                nc.vector.tensor_tensor(ts2[:], ts2[:], tsh[:, 0:1], ALU.add)
                nc.vector.tensor_tensor(ts2[:], ts2[:], tsh[:, 1:2], ALU.add)
                nc.sync.dma_start(SCORE[:], ts2[:])
    nc.compile()
    return nc


# ---------------------------------------------------------------------------
# host side
# ---------------------------------------------------------------------------

def prep_inputs(cfg, sentence, W_ih_f, W_hh_f, b_f, W_ih_b, W_hh_b, b_b,
                W_lin, b_lin, transitions):
    d = _derive(cfg)
    T, F, HH, W = cfg["T"], cfg["F"], cfg["HH"], cfg["W"]
    x = np.ascontiguousarray(sentence[:, 0, :], dtype=np.float32)

    def f16(a):
        return a.astype(np.float16)

    def core_inputs(seq, W_ih, W_hh, b, wl_half):
        outs = []
        waug = np.zeros((d["KAUG"], d["H4"]), np.float32)
        waug[0:F] = W_ih.T.astype(np.float32)
        waug[F] = b.astype(np.float32)
        wlh = f16(wl_half)
        wll = f16(wl_half - wlh.astype(np.float32))
        for q in range(4):
            s0 = q * d["TCORE"] - W
            rows = np.zeros((d["RPAD"], d["KAUG"]), np.float32)
            lo = max(s0, 0)
            rows[lo - s0:d["RROWS"], 0:F] = seq[lo:s0 + d["RROWS"]]
            rows[lo - s0:d["RROWS"], F] = 1.0
            outs.append({
                "xT": np.ascontiguousarray(rows.T),
                "Wih": waug,
                "Whh": np.ascontiguousarray(f16(W_hh.T)),
                "WlinHi": np.ascontiguousarray(wlh),
                "WlinLo": np.ascontiguousarray(wll),
            })
        return outs

    wlf = W_lin[:, 0:HH].T.astype(np.float32)      # [HH, 10]
    wlb = W_lin[:, HH:].T.astype(np.float32)
    cores = (core_inputs(x, W_ih_f, W_hh_f, b_f, wlf)
             + core_inputs(x[::-1].copy(), W_ih_b, W_hh_b, b_b, wlb))

    blin = np.zeros((16, 1), np.float32)
    blin[0:TAGS, 0] = b_lin
    trt = np.ascontiguousarray(
        transitions.T.astype(np.float32).reshape(1, 100))
    stopv = transitions[STOP, :].astype(np.float32).reshape(1, TAGS)
    idf = np.full((TAGS, TAGS), NEG, np.float32)
    np.fill_diagonal(idf, 0.0)
    idf = idf.reshape(1, 100)
    idb = np.eye(cfg["B"], dtype=np.float16)
    for m in cores:
        m["blin"] = blin
        m["transTflat"] = trt
        m["stopv"] = np.ascontiguousarray(stopv)
        m["identflat"] = idf
        m["identB"] = idb
    return cores


def kernel(**inputs):
    from concourse.bass_utils import run_bass_kernel_spmd
    cfg = CFG
    in_maps = prep_inputs(cfg, **{k: np.asarray(v) for k, v in inputs.items()})
    nc = build_nc(cfg)
    res = run_bass_kernel_spmd(nc, in_maps, list(range(NCORES)))
    r0 = res.results[0]
    score = np.float32(r0["score"][0, 0])
    path = r0["best_path"].reshape(-1).astype(np.int32)
    return score, path


# revision 22
# speedup vs baseline: 1.3887x; 1.3546x over previous
"""BiLSTM + CRF Viterbi decode on 8 trn2 NeuronCores (Bass/Tile, SPMD).

Strategy:
  - cores 0-3: forward LSTM over sequence quarters; cores 4-7: backward LSTM
    run as a forward LSTM over the host-reversed sequence. One SPMD program.
  - the serial recurrence is broken with chunked restart: each core runs
    B chunks of length L as a batch with a W-step warmup halo (forget-gate
    contraction makes the halo error ~1e-15 at W=64).
  - input projection xp = [X|mask] @ [W_ih.T; b] in fp32r, staged in DRAM;
    recurrence matmuls in fp16 (stationary h^T via DMA-transpose, moving
    W_hh^T, fp32 PSUM); gate math fp32 on ACT/DVE.
  - feats = h @ W_lin.T with fp16 hi/lo weights; partial feats AllGather'd
    and assembled on every core.
  - Viterbi via normalized max-plus segmented scan (prefix + suffix);
    path[t] = argmax(alpha_t + beta_t), replicated on every core.
"""

import numpy as np

CFG = dict(T=4096, F=2048, HH=1024, B=64, L=16, W=64)
TAGS, START, STOP = 10, 8, 9
NCORES = 8
NEG = -1e9


def _derive(cfg):
    d = dict(cfg)
    d["H4"] = 4 * cfg["HH"]
    d["KH"] = cfg["HH"] // 128
    d["NCH"] = d["H4"] // 512
    d["STEPS"] = cfg["W"] + cfg["L"]
    d["TCORE"] = cfg["T"] // 4
    assert cfg["B"] * cfg["L"] == d["TCORE"]
    d["KAUG"] = cfg["F"] + 128
    d["KA"] = d["KAUG"] // 128
    d["RROWS"] = d["TCORE"] + cfg["W"]
    d["RPAD"] = ((d["RROWS"] + 127) // 128) * 128
    d["RB"] = d["RPAD"] // 128
    d["S"] = cfg["T"] // 128
    return d


# ---------------------------------------------------------------------------
# device program
# ---------------------------------------------------------------------------

def build_nc(cfg):
    import concourse.bacc as bacc
    import concourse.mybir as mybir
    import concourse.tile as tile

    d = _derive(cfg)
    T, F, HH, B, L, W = (cfg[k] for k in ("T", "F", "HH", "B", "L", "W"))
    H4, KH, NCH, STEPS = d["H4"], d["KH"], d["NCH"], d["STEPS"]
    KA, RPAD, RB, S, TCORE = d["KA"], d["RPAD"], d["RB"], d["S"], d["TCORE"]
    dt = mybir.dt
    AF = mybir.ActivationFunctionType
    ALU = mybir.AluOpType
    AX = mybir.AxisListType

    nc = bacc.Bacc(None, target_bir_lowering=False, num_devices=NCORES)

    XT = nc.dram_tensor("xT", [d["KAUG"], RPAD], dt.float32, kind="ExternalInput")
    WIH = nc.dram_tensor("Wih", [d["KAUG"], H4], dt.float32, kind="ExternalInput")
    WHH = nc.dram_tensor("Whh", [HH, H4], dt.float16, kind="ExternalInput")
    WLH = nc.dram_tensor("WlinHi", [HH, TAGS], dt.float16, kind="ExternalInput")
    WLL = nc.dram_tensor("WlinLo", [HH, TAGS], dt.float16, kind="ExternalInput")
    BLIN = nc.dram_tensor("blin", [16, 1], dt.float32, kind="ExternalInput")
    TRT = nc.dram_tensor("transTflat", [1, 100], dt.float32, kind="ExternalInput")
    STV = nc.dram_tensor("stopv", [1, TAGS], dt.float32, kind="ExternalInput")
    IDF = nc.dram_tensor("identflat", [1, 100], dt.float32, kind="ExternalInput")
    IDB = nc.dram_tensor("identB", [cfg["B"], cfg["B"]], dt.float16, kind="ExternalInput")
    SCORE = nc.dram_tensor("score", [1, 1], dt.float32, kind="ExternalOutput")
    BPATH = nc.dram_tensor("best_path", [1, T], dt.int32, kind="ExternalOutput")
    DBGF = nc.dram_tensor("dbg_feats", [TAGS, T], dt.float32, kind="ExternalOutput")
    DBGQ = nc.dram_tensor("dbg_xq", [3 * cfg["B"], H4], dt.float32, kind="ExternalOutput")
    DBGQ2 = nc.dram_tensor("dbg_xq2", [d["STEPS"], H4], dt.float32, kind="ExternalOutput")
    DBGH = nc.dram_tensor("dbg_hist0", [128, d["TCORE"]], dt.float16, kind="ExternalOutput")
    DBGH16 = nc.dram_tensor("dbg_h16", [4 * cfg["B"], HH], dt.float16, kind="ExternalOutput")

    with tile.TileContext(nc) as tc:
        with tc.tile_pool(name="dram", bufs=1, space="DRAM") as dpool:
            import os as _os0
            XP_FLAT = bool(_os0.environ.get("XP_FLAT"))
            xp_d = (dpool.tile([RPAD, H4], dt.float32, tag="xp_d", name="xp_d")
                    if XP_FLAT else
                    dpool.tile([L, RPAD // L, H4], dt.float32, tag="xp_d", name="xp_d"))
            ag_in = dpool.tile([16, TCORE], dt.float32, tag="ag_in")
            ag_out = dpool.tile([16 * NCORES, TCORE], dt.float32,
                                addr_space="Shared", tag="ag_out")
            ftT_d = dpool.tile([TAGS, T], dt.float32, tag="ftT_d")
            sp_d = dpool.tile([128, 100], dt.float32, tag="sp_d")
            ss_d = dpool.tile([128, 1], dt.float32, tag="ss_d")
            g16_d = dpool.tile([16, 100], dt.float32, tag="g16_d")
            gs16_d = dpool.tile([16, 1], dt.float32, tag="gs16_d")
            e16_d = dpool.tile([16, 100], dt.float32, tag="e16_d")
            es16_d = dpool.tile([16, 1], dt.float32, tag="es16_d")
            grid_d = dpool.tile([128, 100], dt.float32, tag="grid_d")
            grs_d = dpool.tile([128, 1], dt.float32, tag="grs_d")

            # ================= Phase A: xp GEMM =================
            with (
                nc.named_scope("phA"),
                tc.tile_pool(name="sbA", bufs=1) as sba,
                tc.tile_pool(name="wpan", bufs=2) as wpan,
                tc.tile_pool(name="psA", bufs=4, space="PSUM") as psa,
                tc.tile_pool(name="outA", bufs=4) as outa,
            ):
                xp_writes = []
                xt = sba.tile([128, KA, RPAD], dt.float32r, tag="xt")
                nc.sync.dma_start(
                    xt[:],
                    XT[:].rearrange("(k p) r -> p k r", p=128).bitcast(dt.float32r))
                for j in range(NCH):
                    wp = wpan.tile([128, KA, 512], dt.float32r, tag="wp")
                    nc.sync.dma_start(
                        wp[:],
                        WIH[:].rearrange("(k p) n -> p k n", p=128)
                        [:, :, 512 * j:512 * (j + 1)].bitcast(dt.float32r))
                    for rb in range(RB):
                        pj = psa.tile([128, 512], dt.float32, tag="pj")
                        for k in range(KA):
                            nc.tensor.matmul(pj[:], xt[:, k, 128 * rb:128 * (rb + 1)],
                                             wp[:, k, :], start=(k == 0),
                                             stop=(k == KA - 1))
                        ot = outa.tile([128, 512], dt.float32, tag="ot")
                        nc.vector.tensor_copy(ot[:], pj[:])
                        PPER = 128 // L
                        if XP_FLAT:
                            wi = nc.sync.dma_start(
                                xp_d[128 * rb:128 * (rb + 1), 512 * j:512 * (j + 1)], ot[:])
                        else:
                            wi = nc.sync.dma_start(
                                xp_d[:, PPER * rb:PPER * (rb + 1), 512 * j:512 * (j + 1)]
                                .rearrange("m j c -> j m c"), ot[:])
                        xp_writes.append(wi.ins)

            # ================= Phase B + C =================
            with nc.named_scope("phB"), tc.tile_pool(name="sbB", bufs=1) as sbb:
                whh = sbb.tile([128, KH, H4], dt.float16, tag="whh")
                nc.sync.dma_start(whh[:], WHH[:].rearrange("(k p) n -> p k n", p=128))
                hist = [sbb.tile([128, TCORE], dt.float16, tag=f"hist{k}", name=f"hist{k}")
                        for k in range(KH)]
                halo = [sbb.tile([128, 2 * B], dt.float16, tag=f"halo{k}", name=f"halo{k}")
                        for k in range(KH)]
                zer = sbb.tile([128, B], dt.float16, tag="zer")
                nc.vector.memset(zer[:], 0.0)
                cst = sbb.tile([B, HH], dt.float32, tag="cst")
                nc.vector.memset(cst[:], 0.0)
                sgi = sbb.tile([B, HH], dt.float32, tag="sgi")
                sgf = sbb.tile([B, HH], dt.float32, tag="sgf")
                sgg = sbb.tile([B, HH], dt.float32, tag="sgg")
                sgo = sbb.tile([B, HH], dt.float32, tag="sgo")
                tnc = sbb.tile([B, HH], dt.float32, tag="tnc")
                tm1 = sbb.tile([B, HH], dt.float32, tag="tm1")
                tm2 = sbb.tile([B, HH], dt.float32, tag="tm2")
                h16 = sbb.tile([B, HH], dt.float16, tag="h16")
                gsb = [sbb.tile([B, 512], dt.float32, tag=f"gsb{n}", name=f"gsb{n}") for n in range(NCH)]
                sgate = {0: sgi, 1: sgf, 2: sgg, 3: sgo}

                def hsrc(t, k):
                    if t < 0:
                        return zer[:]
                    if t < W:
                        return halo[k][:, (t % 2) * B:(t % 2 + 1) * B]
                    return hist[k][:, (t - W) * B:(t - W + 1) * B]

                with (
                    tc.tile_pool(name="xqp", bufs=2) as xqp,
                    tc.tile_pool(name="psB", bufs=1, space="PSUM") as psb,
                ):
                    import os as _os2
                    NPG = NCH if _os2.environ.get("NPG_FULL") else min(4, NCH)
                    pg = [psb.tile([B, 512], dt.float32, tag=f"pg{n}", name=f"pg{n}")
                          for n in range(NPG)]
                    ptx = ([] if _os2.environ.get("NPG_FULL") else
                           [psb.tile([128, 4 * B], dt.float16, tag=f"ptx{i}", name=f"ptx{i}")
                            for i in range(2)])
                    idb = sbb.tile([B, B], dt.float16, tag="idb")
                    nc.sync.dma_start(idb[:], IDB[:])
                    from concourse.tile_rust import add_dep_helper as _adh
                    fence_t = sbb.tile([1, 1], dt.float32, tag="fence_t")
                    fi = nc.vector.memset(fence_t[:], 0.0)
                    for wi_ in xp_writes:
                        _adh(fi.ins, wi_, sync=True, reason="xp fence in")
                    for t in range(STEPS):
                        xq = xqp.tile([B, H4], dt.float32, tag="xq")
                        ri = (nc.sync.dma_start(xq[:], xp_d[t:t + B * L:L, :])
                              if XP_FLAT else
                              nc.sync.dma_start(xq[:], xp_d[t % L, t // L:t // L + B, :]))
                        _adh(ri.ins, fi.ins, sync=True, reason="xp fence out")
                        for n in range(NCH):
                            for k in range(KH):
                                nc.tensor.matmul(pg[n % NPG][:], hsrc(t - 1, k),
                                                 whh[:, k, 512 * n:512 * (n + 1)],
                                                 start=(k == 0), stop=(k == KH - 1))
                            nc.vector.tensor_tensor(gsb[n][:], pg[n % NPG][:],
                                                    xq[:, 512 * n:512 * (n + 1)],
                                                    ALU.add)
                            c0, c1 = 512 * n, 512 * (n + 1)
                            for g in range(c0 // HH, (c1 - 1) // HH + 1):
                                lo, hi = max(c0, g * HH), min(c1, (g + 1) * HH)
                                fn = AF.Tanh if g == 2 else AF.Sigmoid
                                nc.scalar.activation(
                                    sgate[g][:, lo - g * HH:hi - g * HH],
                                    gsb[n][:, lo - c0:hi - c0], fn)
                        nc.vector.tensor_tensor(tm1[:], sgf[:], cst[:], ALU.mult)
                        nc.vector.tensor_tensor(tm2[:], sgi[:], sgg[:], ALU.mult)
                        nc.vector.tensor_tensor(cst[:], tm1[:], tm2[:], ALU.add)
                        nc.scalar.activation(tnc[:], cst[:], AF.Tanh)
                        nc.vector.tensor_tensor(h16[:], sgo[:], tnc[:], ALU.mult)
                        if t in (0, 1, 17):
                            di = (0, 1, 17).index(t)
                            nc.sync.dma_start(DBGQ[di * B:(di + 1) * B, :], xq[:])
                        nc.sync.dma_start(DBGQ2[t:t + 1, :], xq[0:1, :])
                        if t in (0, 1, 2, 40):
                            di2 = (0, 1, 2, 40).index(t)
                            nc.sync.dma_start(DBGH16[di2 * B:(di2 + 1) * B, :], h16[:])
                        import os as _os
                        for k in range(KH):
                            dst = (halo[k][:, (t % 2) * B:(t % 2 + 1) * B] if t < W
                                   else hist[k][:, (t - W) * B:(t - W + 1) * B])
                            if _os.environ.get("NO_PE_T"):
                                nc.sync.dma_start_transpose(
                                    dst, h16[:, 128 * k:128 * (k + 1)])
                            else:
                                pslot = ptx[(k // 4) % 2][:, (k % 4) * B:(k % 4 + 1) * B]
                                nc.tensor.transpose(pslot,
                                                    h16[:, 128 * k:128 * (k + 1)],
                                                    idb[:])
                                nc.vector.tensor_copy(dst, pslot)

                nc.sync.dma_start(DBGH[:], hist[0][:])
                # ---- Phase C: partial feats ----
                with (
                    nc.named_scope("phC"),
                    tc.tile_pool(name="sbC", bufs=1) as sbc,
                    tc.tile_pool(name="psC", bufs=2, space="PSUM") as psc,
                ):
                    wlh = sbc.tile([128, KH, TAGS], dt.float16, tag="wlh")
                    wll = sbc.tile([128, KH, TAGS], dt.float16, tag="wll")
                    nc.sync.dma_start(wlh[:], WLH[:].rearrange("(k p) n -> p k n", p=128))
                    nc.sync.dma_start(wll[:], WLL[:].rearrange("(k p) n -> p k n", p=128))
                    fpart = sbc.tile([16, TCORE], dt.float32, tag="fpart")
                    nc.vector.memset(fpart[:], 0.0)
                    FW = min(512, TCORE)
                    for n2 in range(TCORE // FW):
                        pf = psc.tile([TAGS, FW], dt.float32, tag="pf")
                        first = True
                        for k in range(KH):
                            for wl in (wlh, wll):
                                nc.tensor.matmul(
                                    pf[:], wl[:, k, :],
                                    hist[k][:, FW * n2:FW * (n2 + 1)],
                                    start=first,
                                    stop=(k == KH - 1 and wl is wll))
                                first = False
                        nc.vector.tensor_copy(
                            fpart[0:TAGS, FW * n2:FW * (n2 + 1)], pf[:])
                    nc.sync.dma_start(ag_in[:], fpart[:])

            nc.gpsimd.collective_compute(
                "AllGather", mybir.AluOpType.bypass,
                replica_groups=[list(range(NCORES))],
                ins=[ag_in[:].opt()], outs=[ag_out[:].opt()])

            # ================= Phase D: assemble feats =================
            with nc.named_scope("phD"), tc.tile_pool(name="sbD", bufs=1) as sbd:
                ag_c = [sbd.tile([16, TCORE], dt.float32, tag=f"ag{c}", name=f"ag{c}")
                        for c in range(NCORES)]
                for c in range(NCORES):
                    nc.sync.dma_start(ag_c[c][:], ag_out[16 * c:16 * (c + 1), :])
                blin_sb = sbd.tile([16, 1], dt.float32, tag="blin_sb")
                nc.sync.dma_start(blin_sb[:], BLIN[:])
                ftT = sbd.tile([TAGS, T], dt.float32, tag="ftT")
                for q in range(4):
                    fwd = (ag_c[q][0:TAGS, :]
                           .rearrange("p (t b) -> p b t", b=B))
                    bwd = (ag_c[7 - q][0:TAGS, ::-1]
                           .rearrange("p (t b) -> p b t", b=B))
                    nc.vector.tensor_tensor(
                        ftT[:, TCORE * q:TCORE * (q + 1)]
                        .rearrange("p (b t) -> p b t", b=B),
                        fwd, bwd, ALU.add)
                nc.vector.tensor_scalar_add(ftT[:], ftT[:], blin_sb[0:TAGS, 0:1])
                nc.sync.dma_start(DBGF[:], ftT[:])
                nc.sync.dma_start(ftT_d[:], ftT[:])

            # ================= Phase E: Viterbi =================
            with nc.named_scope("phE"), tc.tile_pool(name="sbE", bufs=1) as sbe:
                trt = sbe.tile([128, 100], dt.float32, tag="trt")
                stv = sbe.tile([128, TAGS], dt.float32, tag="stv")
                idf = sbe.tile([16, 100], dt.float32, tag="idf")
                zro = sbe.tile([16, 1], dt.float32, tag="zro")
                nc.vector.memset(zro[:], 0.0)
                nc.sync.dma_start(trt[0:1, :], TRT[:])
                nc.sync.dma_start(stv[0:1, :], STV[:])
                nc.sync.dma_start(idf[0:1, :], IDF[:])
                p = 1
                while p < 128:
                    q = min(p, 128 - p)
                    nc.sync.dma_start(trt[p:p + q, :], trt[0:q, :])
                    nc.sync.dma_start(stv[p:p + q, :], stv[0:q, :])
                    if p < 16:
                        q2 = min(p, 16 - p)
                        nc.sync.dma_start(idf[p:p + q2, :], idf[0:q2, :])
                    p *= 2

                ftseg = sbe.tile([128, TAGS, S], dt.float32, tag="ftseg")
                nc.sync.dma_start(ftseg[:],
                                  ftT_d[:].rearrange("n (q r) -> q n r", r=S))
                leaf = sbe.tile([128, S * 100], dt.float32, tag="leaf")
                nc.vector.tensor_tensor(
                    leaf[:].rearrange("q (r i n) -> q r i n", i=TAGS, n=TAGS),
                    trt[:].rearrange("q (i n) -> q i n", i=TAGS)
                    .unsqueeze(1).broadcast_to([128, S, TAGS, TAGS]),
                    ftseg[:].rearrange("q n r -> q r n")
                    .unsqueeze(2).broadcast_to([128, S, TAGS, TAGS]),
                    ALU.add)

                csc = sbe.tile([128, 1000], dt.float32, tag="csc")
                mx1 = sbe.tile([128, 1], dt.float32, tag="mx1")

                def compose(av, bv, ov, shin, shout, P=128):
                    nc.vector.tensor_tensor(
                        csc[0:P, :].rearrange("q (i n k) -> q i n k",
                                              i=TAGS, n=TAGS),
                        av.rearrange("q (i k) -> q i k", i=TAGS)
                        .unsqueeze(2).broadcast_to([P, TAGS, TAGS, TAGS]),
                        bv.rearrange("q (k n) -> q n k", k=TAGS)
                        .unsqueeze(1).broadcast_to([P, TAGS, TAGS, TAGS]),
                        ALU.add)
                    nc.vector.tensor_reduce(
                        ov, csc[0:P, :].rearrange("q (in k) -> q in k", k=TAGS),
                        AX.X, ALU.max)
                    nc.vector.tensor_reduce(mx1[0:P, :], ov, AX.X, ALU.max)
                    nc.vector.tensor_scalar_sub(ov, ov, mx1[0:P, 0:1])
                    if len(shin) > 1:
                        nc.vector.tensor_tensor(shout, shin[0], shin[1], ALU.add)
                        nc.vector.tensor_tensor(shout, shout, mx1[0:P, :], ALU.add)
                    else:
                        nc.vector.tensor_tensor(shout, shin[0], mx1[0:P, :], ALU.add)

                def seg_scan(direction, tag):
                    pref = sbe.tile([128, S * 100], dt.float32, tag="pref" + tag)
                    psh = sbe.tile([128, S], dt.float32, tag="psh" + tag)
                    nc.vector.memset(psh[:], 0.0)
                    rng = list(range(S)) if direction > 0 else list(range(S - 1, -1, -1))
                    r0 = rng[0]
                    nc.vector.tensor_copy(pref[:, r0 * 100:(r0 + 1) * 100],
                                          leaf[:, r0 * 100:(r0 + 1) * 100])
                    for r in rng[1:]:
                        prev = r - direction
                        lv = leaf[:, r * 100:(r + 1) * 100]
                        pv = pref[:, prev * 100:(prev + 1) * 100]
                        av, bv = (pv, lv) if direction > 0 else (lv, pv)
                        compose(av, bv, pref[:, r * 100:(r + 1) * 100],
                                [psh[:, prev:prev + 1]], psh[:, r:r + 1])
                    return pref, psh

                def cross_scan(pref, psh, direction, tag):
                    """exclusive scan over the 128 segment products."""
                    last = S - 1 if direction > 0 else 0
                    nc.sync.dma_start(sp_d[:], pref[:, last * 100:(last + 1) * 100])
                    nc.sync.dma_start(ss_d[:], psh[:, last:last + 1])
                    l1 = sbe.tile([16, 800], dt.float32, tag="l1" + tag)
                    l1s = sbe.tile([16, 8], dt.float32, tag="l1s" + tag)
                    nc.sync.dma_start(l1[:], sp_d[:].rearrange("(g j) e -> g (j e)", j=8))
                    nc.sync.dma_start(l1s[:], ss_d[:].rearrange("(g j) e -> g (j e)", j=8))
                    rng = list(range(8)) if direction > 0 else list(range(7, -1, -1))
                    for j in rng[1:]:
                        prev = j - direction
                        sv = l1[:, j * 100:(j + 1) * 100]
                        pv = l1[:, prev * 100:(prev + 1) * 100]
                        av, bv = (pv, sv) if direction > 0 else (sv, pv)
                        compose(av, bv, sv, [l1s[:, prev:prev + 1], l1s[:, j:j + 1]],
                                l1s[:, j:j + 1], P=16)
                    lastj = 7 if direction > 0 else 0
                    nc.sync.dma_start(g16_d[:], l1[:, lastj * 100:(lastj + 1) * 100])
                    nc.sync.dma_start(gs16_d[:], l1s[:, lastj:lastj + 1])
                    l2 = sbe.tile([1, 1600], dt.float32, tag="l2" + tag)
                    l2s = sbe.tile([1, 16], dt.float32, tag="l2s" + tag)
                    nc.sync.dma_start(l2[0:1, :],
                                      g16_d[:].rearrange("g e -> (g e)").unsqueeze(0))
                    nc.sync.dma_start(l2s[0:1, :],
                                      gs16_d[:].rearrange("g e -> (g e)").unsqueeze(0))
                    rng2 = list(range(16)) if direction > 0 else list(range(15, -1, -1))
                    for j in rng2[1:]:
                        prev = j - direction
                        sv = l2[:, j * 100:(j + 1) * 100]
                        pv = l2[:, prev * 100:(prev + 1) * 100]
                        av, bv = (pv, sv) if direction > 0 else (sv, pv)
                        compose(av, bv, sv, [l2s[:, prev:prev + 1], l2s[:, j:j + 1]],
                                l2s[:, j:j + 1], P=1)
                    # exclusive level-2 (group) prefixes -> e16_d (+ shifts)
                    if direction > 0:
                        nc.sync.dma_start(e16_d[1:16, :], l2[0:1, 0:1500])
                        nc.sync.dma_start(es16_d[1:16, :], l2s[0:1, 0:15])
                        nc.sync.dma_start(e16_d[0:1, :], idf[0:1, :])
                        nc.sync.dma_start(es16_d[0:1, :], zro[0:1, :])
                    else:
                        nc.sync.dma_start(e16_d[0:15, :], l2[0:1, 100:1600])
                        nc.sync.dma_start(es16_d[0:15, :], l2s[0:1, 1:16])
                        nc.sync.dma_start(e16_d[15:16, :], idf[0:1, :])
                        nc.sync.dma_start(es16_d[15:16, :], zro[0:1, :])
                    # shifted within-group prefixes -> grid_d (+ shifts)
                    gv = grid_d[:].rearrange("(h s) e -> h s e", s=8)
                    gsv = grs_d[:].rearrange("(h s) e -> h s e", s=8)
                    if direction > 0:
                        nc.sync.dma_start(gv[:, 1:8, :], l1[:, 0:700])
                        nc.sync.dma_start(gsv[:, 1:8, :], l1s[:, 0:7])
                        nc.sync.dma_start(gv[:, 0:1, :], idf[:, :].unsqueeze(1))
                        nc.sync.dma_start(gsv[:, 0:1, :], zro[:, :].unsqueeze(1))
                    else:
                        nc.sync.dma_start(gv[:, 0:7, :], l1[:, 100:800])
                        nc.sync.dma_start(gsv[:, 0:7, :], l1s[:, 1:8])
                        nc.sync.dma_start(gv[:, 7:8, :], idf[:, :].unsqueeze(1))
                        nc.sync.dma_start(gsv[:, 7:8, :], zro[:, :].unsqueeze(1))
                    # materialize exc [128, 100]
                    arow = sbe.tile([128, 100], dt.float32, tag="arow" + tag)
                    ars = sbe.tile([128, 1], dt.float32, tag="ars" + tag)
                    nc.sync.dma_start(
                        arow[:], e16_d[:].unsqueeze(1).broadcast_to([16, 8, 100]))
                    nc.sync.dma_start(
                        ars[:], es16_d[:].unsqueeze(1).broadcast_to([16, 8, 1]))
                    brow = sbe.tile([128, 100], dt.float32, tag="brow" + tag)
                    brs = sbe.tile([128, 1], dt.float32, tag="brs" + tag)
                    nc.sync.dma_start(brow[:], grid_d[:])
                    nc.sync.dma_start(brs[:], grs_d[:])
                    exc = sbe.tile([128, 100], dt.float32, tag="exc" + tag)
                    excs = sbe.tile([128, 1], dt.float32, tag="excs" + tag)
                    av, bv = (arow[:], brow[:]) if direction > 0 else (brow[:], arow[:])
                    compose(av, bv, exc[:], [ars[:], brs[:]], excs[:])
                    return exc, excs

                prefP, pshP = seg_scan(+1, "P")
                excP, excPs = cross_scan(prefP, pshP, +1, "P")
                prefS, pshS = seg_scan(-1, "S")
                excS, excSs = cross_scan(prefS, pshS, -1, "S")

                # ---- alpha: a[t, n] = max_k excP[q][START, k] + prefP[q,r][k, n]
                cbig = sbe.tile([128, S * 100], dt.float32, tag="cbig")
                aseg = sbe.tile([128, S * TAGS], dt.float32, tag="aseg")
                nc.vector.tensor_tensor(
                    cbig[:].rearrange("q (r n k) -> q r n k", n=TAGS, k=TAGS),
                    excP[:, START * TAGS:(START + 1) * TAGS]
                    .unsqueeze(1).unsqueeze(2).broadcast_to([128, S, TAGS, TAGS]),
                    prefP[:].rearrange("q (r k n) -> q r n k", k=TAGS, n=TAGS),
                    ALU.add)
                nc.vector.tensor_reduce(
                    aseg[:], cbig[:].rearrange("q (rn k) -> q rn k", k=TAGS),
                    AX.X, ALU.max)

                # ---- z[q][m] = max_k excS[q][m, k] + stopv[k]
                zq = sbe.tile([128, TAGS], dt.float32, tag="zq")
                nc.vector.tensor_tensor(
                    csc[:, 0:100].rearrange("q (m k) -> q m k", m=TAGS),
                    stv[:].unsqueeze(1).broadcast_to([128, TAGS, TAGS]),
                    excS[:].rearrange("q (m k) -> q m k", m=TAGS),
                    ALU.add)
                nc.vector.tensor_reduce(
                    zq[:], csc[:, 0:100].rearrange("q (m k) -> q m k", m=TAGS),
                    AX.X, ALU.max)

                # ---- beta: b[t, i] = max_m prefS[q, r+1][i, m] + z[q][m]
                bseg = sbe.tile([128, S * TAGS], dt.float32, tag="bseg")
                nc.vector.tensor_tensor(
                    cbig[:, 0:(S - 1) * 100]
                    .rearrange("q (r i m) -> q r i m", i=TAGS, m=TAGS),
                    prefS[:, 100:].rearrange("q (r i m) -> q r i m", i=TAGS, m=TAGS),
                    zq[:].unsqueeze(1).unsqueeze(2)
                    .broadcast_to([128, S - 1, TAGS, TAGS]),
                    ALU.add)
                nc.vector.tensor_reduce(
                    bseg[:, 0:(S - 1) * TAGS],
                    cbig[:, 0:(S - 1) * 100].rearrange("q (ri m) -> q ri m", m=TAGS),
                    AX.X, ALU.max)
                nc.vector.tensor_copy(bseg[:, (S - 1) * TAGS:S * TAGS], zq[:])

                # ---- path = argmax(alpha + beta) ----
                sc = sbe.tile([128, S * TAGS], dt.float32, tag="sc")
                nc.vector.tensor_tensor(sc[:], aseg[:], bseg[:], ALU.add)
                mv8 = sbe.tile([128, 8], dt.float32, tag="mv8")
                mi8 = sbe.tile([128, 8], dt.uint32, tag="mi8")
                path = sbe.tile([128, S], dt.int32, tag="path")
                for r in range(S):
                    nc.vector.max_with_indices(mv8[:], mi8[:],
                                               sc[:, r * TAGS:(r + 1) * TAGS])
                    nc.vector.tensor_copy(path[:, r:r + 1],
                                          mi8[:, 0:1].bitcast(dt.int32))
                nc.sync.dma_start(
                    BPATH[0:1, :].rearrange("p (q r) -> (p q) r", q=128), path[:])

                # ---- score = max(alpha_{T-1} + stopv) + shifts ----
                ts0 = sbe.tile([1, TAGS], dt.float32, tag="ts0")
                tsh = sbe.tile([1, 2], dt.float32, tag="tsh")
                nc.sync.dma_start(ts0[:], aseg[127:128, (S - 1) * TAGS:S * TAGS])
                nc.sync.dma_start(tsh[:, 0:1], excPs[127:128, :])
                nc.sync.dma_start(tsh[:, 1:2], pshP[127:128, S - 1:S])
                ts1 = sbe.tile([1, TAGS], dt.float32, tag="ts1")
                ts2 = sbe.tile([1, 1], dt.float32, tag="ts2")
                nc.vector.tensor_tensor(ts1[:], ts0[:], stv[0:1, :], ALU.add)
                nc.vector.tensor_reduce(ts2[:], ts1[:], AX.X, ALU.max)
                nc.vector.tensor_tensor(ts2[:], ts2[:], tsh[:, 0:1], ALU.add)
                nc.vector.tensor_tensor(ts2[:], ts2[:], tsh[:, 1:2], ALU.add)
                nc.sync.dma_start(SCORE[:], ts2[:])
    nc.compile()
    return nc


# ---------------------------------------------------------------------------
# host side
# ---------------------------------------------------------------------------

def prep_inputs(cfg, sentence, W_ih_f, W_hh_f, b_f, W_ih_b, W_hh_b, b_b,
                W_lin, b_lin, transitions):
    d = _derive(cfg)
    T, F, HH, W = cfg["T"], cfg["F"], cfg["HH"], cfg["W"]
    x = np.ascontiguousarray(sentence[:, 0, :], dtype=np.float32)

    def f16(a):
        return a.astype(np.float16)

    def core_inputs(seq, W_ih, W_hh, b, wl_half):
        outs = []
        waug = np.zeros((d["KAUG"], d["H4"]), np.float32)
        waug[0:F] = W_ih.T.astype(np.float32)
        waug[F] = b.astype(np.float32)
        wlh = f16(wl_half)
        wll = f16(wl_half - wlh.astype(np.float32))
        for q in range(4):
            s0 = q * d["TCORE"] - W
            rows = np.zeros((d["RPAD"], d["KAUG"]), np.float32)
            lo = max(s0, 0)
            rows[lo - s0:d["RROWS"], 0:F] = seq[lo:s0 + d["RROWS"]]
            rows[lo - s0:d["RROWS"], F] = 1.0
            outs.append({
                "xT": np.ascontiguousarray(rows.T),
                "Wih": waug,
                "Whh": np.ascontiguousarray(f16(W_hh.T)),
                "WlinHi": np.ascontiguousarray(wlh),
                "WlinLo": np.ascontiguousarray(wll),
            })
        return outs

    wlf = W_lin[:, 0:HH].T.astype(np.float32)      # [HH, 10]
    wlb = W_lin[:, HH:].T.astype(np.float32)
    cores = (core_inputs(x, W_ih_f, W_hh_f, b_f, wlf)
             + core_inputs(x[::-1].copy(), W_ih_b, W_hh_b, b_b, wlb))

    blin = np.zeros((16, 1), np.float32)
    blin[0:TAGS, 0] = b_lin
    trt = np.ascontiguousarray(
        transitions.T.astype(np.float32).reshape(1, 100))
    stopv = transitions[STOP, :].astype(np.float32).reshape(1, TAGS)
    idf = np.full((TAGS, TAGS), NEG, np.float32)
    np.fill_diagonal(idf, 0.0)
    idf = idf.reshape(1, 100)
    idb = np.eye(cfg["B"], dtype=np.float16)
    for m in cores:
        m["blin"] = blin
        m["transTflat"] = trt
        m["stopv"] = np.ascontiguousarray(stopv)
        m["identflat"] = idf
        m["identB"] = idb
    return cores


def kernel(**inputs):
    from concourse.bass_utils import run_bass_kernel_spmd
    cfg = CFG
    in_maps = prep_inputs(cfg, **{k: np.asarray(v) for k, v in inputs.items()})
    nc = build_nc(cfg)
    res = run_bass_kernel_spmd(nc, in_maps, list(range(NCORES)))
    r0 = res.results[0]
    score = np.float32(r0["score"][0, 0])
    path = r0["best_path"].reshape(-1).astype(np.int32)
    return score, path
